# revision 4
# baseline (speedup 1.0000x reference)
"""SSIM loss kernel for Trainium2, v18: s/d basis + custom DVE ops.

Per core: 6 planes of 512x512. Host casts x,y to bf16 and stacks.
Per plane:
  sm=x+y, dm=x-y (TT, split DVE/Pool by column knob)
  scan1: stock merged scan over [sm,dm] -> hs,hd        (DVE)
  SQSCAN: custom scan h += Src0^2-Src1^2 over [sm,dm] -> hss,hdd (DVE)
  42 matmuls/plane: PA=w.hs, PB=w.hd, T=wb.hss-wb.hdd, V2=wb.(hss+hdd)
  Act: Square pair (PA,PB)->MS,MD ; Copy+2c2 pair (T,V2)->cT,cV
  m1=MS-MD, m2=MS+MD (TT, engine knobs)
  NUMDEN custom: [num,den] = (M12+2c1)*(CTV-M12) paired   (DVE)
  DIVACC custom: ssim=num*recip_nr1(den), accum -> acc    (DVE)
Host: 1 - sum(acc)/(48*502*502).
"""

import sys
from contextlib import ExitStack

import numpy as np

sys.path.insert(0, "/opt/trn_rl_repo")

import ml_dtypes  # noqa: E402

import concourse.bass as bass  # noqa: E402
import concourse.tile as tile  # noqa: E402
from concourse import bacc, bass_utils, mybir  # noqa: E402
from concourse import dve_ops  # noqa: E402
from concourse.dve_spec import (  # noqa: E402
    AluOp, Bin, C0, C1, Spec, Src0, Src1, lower, scan, sq,
)
from concourse.dve_uop import DveOpSpec  # noqa: E402

F32 = mybir.dt.float32
BF16 = mybir.dt.bfloat16
ALU = mybir.AluOpType
ACTF = mybir.ActivationFunctionType

WIN = 11
IMG = 512
OUT = IMG - WIN + 1  # 502
SEG = WIN + IMG  # 523
NSEG = 4
BUF = NSEG * SEG  # 2092
NPLANE = 6
NCORES = 8
W4 = 4 * OUT  # 2008

C1C = (0.01 * 1.0) ** 2
C2C = (0.03 * 1.0) ** 2
G = 121.0 / 128.0
C1H = np.float32(C1C * G * G)
C2H = np.float32(C2C * G * G)
W_A = float(2.0 ** -7)
W_B = float(121.0 * 2.0 ** -14)
NCLS = 3

_PAIRS = [(0, 0), (0, 1), (1, 1), (1, 2), (2, 2), (2, 3), (3, 3)]
_WIDX = {mk: i for i, mk in enumerate(_PAIRS)}
NP_ = len(_PAIRS)

# ---- knobs ---------------------------------------------------------------
import os as _os


def _env(name, default):
    v = _os.environ.get("V18_" + name)
    if v is None:
        return default
    return eval(v)  # noqa: S307 - trusted local tuning knob


# per-plane: first *_DVE cols of sm/dm on DVE, rest on Pool
SM_DVE = _env("SM_DVE", (BUF, 1046, 1046, 1046, 1046, 1046))
DM_DVE = _env("DM_DVE", (BUF, 0, 0, 0, 0, 0))
# products emitted in the load iteration (Pool gets a stage of slack)
PROD_EARLY = _env("PROD_EARLY", 2)
# per-plane m1/m2 engine: v=DVE, p=Pool (last plane on DVE to cut drain)
M1_ENG = _env("M1_ENG", ("p", "p", "p", "p", "p", "v"))
M2_ENG = _env("M2_ENG", ("p", "p", "p", "p", "p", "v"))
# per-plane: den lane via stock ops on Pool/Act instead of fused NUMDEN
DEN_STOCK = _env("DEN_STOCK", (False,) * 6)
LOAD_SPLIT = _env("LOAD_SPLIT", 0)  # 1: y-load on Act DMA ring
SD_BUFS = _env("SD_BUFS", 3)
# per-plane: tail via Act reciprocal + DVE 2x TT + Act accum (else DIVACC)
TAIL_STOCK = _env("TAIL_STOCK", (True, True, True, True, True, False))
POOL_M12_FIRST = _env("POOL_M12_FIRST", 0)
TAIL_CHUNK = _env("TAIL_CHUNK", 1)  # last plane: per-mblock chain after its act pairs
LOAD_SPLIT0 = _env("LOAD_SPLIT0", 1)  # plane 0 loads on two DMA rings
P0_HALF = _env("P0_HALF", 1)  # plane 0: half-plane loads/products/scans


# ---- custom DVE ops ------------------------------------------------------
def _register(name, spec, subdim=False):
    for op in dve_ops.OPS:
        if op.name == name:
            return op
    shas = {}
    for ver in ("v3", "v4"):
        s = DveOpSpec(name=name, opcode=0, uops=lower(spec, ver=ver),
                      rd1_en=True)
        shas[ver] = s.sha(ver)
    op = dve_ops.DveOp(name, spec, subdim=subdim, uops_sha=shas)
    dve_ops.OPS.append(op)
    dve_ops.CUSTOM_DVE_SPECS[name] = spec
    dve_ops._SUB_OPCODE_FOR_NAME[name] = (
        dve_ops._CUSTOM_DVE_ROW_BASE + len(dve_ops.OPS) - 1
    )
    return op


SQSCAN = _register(
    "SSIM_SQSCAN",
    Spec(
        body=scan(AluOp.ADD, sq(Src0) - sq(Src1)),
        reference=lambda in0, in1, c0, c1, c2: np.cumsum(
            in0.astype(np.float32) ** 2 - in1.astype(np.float32) ** 2,
            axis=-1,
        ),
    ),
)

NUMDEN = _register(
    "SSIM_NUMDEN",
    Spec(
        body=(Src0 + C0) * (Src1 - Src0),
        reference=lambda in0, in1, c0, c1, c2: (
            (in0.astype(np.float32) + c0) * (in1 - in0)
        ),
    ),
)

_not = Bin(AluOp.BITWISE_NOT, Src0, Src0)
_y0 = _not * C0
_y1 = _y0 * (C1 - Src0 * _y0)


def _ref_divacc(in0, in1, c0, c1, c2):
    nx = (~in0.astype(np.float32).view(np.int32)).view(np.float32)
    y0 = nx * c0
    y1 = y0 * (c1 - in0 * y0)
    o = (in1 * y1).astype(np.float32)
    return o, o.reshape(o.shape[0], -1).sum(-1, keepdims=True)


DIVACC = _register(
    "SSIM_DIVACC",
    Spec(body=Src1 * _y1, accum=AluOp.ADD, reference=_ref_divacc),
)
_RC = dve_ops.RECIP_APPROX_FAST_CONSTS


def _build_weights() -> np.ndarray:
    w = np.zeros((NCLS, NP_, 128, 128), dtype=np.float32)
    vals = [W_A, W_B, -W_B]
    for idx, (m, k) in enumerate(_PAIRS):
        for i in range(128):
            for o in range(128):
                d = (128 * k + i) - (128 * m + o)
                if 0 <= d < WIN:
                    for c in range(NCLS):
                        w[c, idx, i, o] = vals[c]
    return np.ascontiguousarray(
        w.transpose(2, 0, 1, 3).reshape(128, NCLS * NP_, 128)
    ).astype(ml_dtypes.bfloat16)


def _act_recip(eng, out, in_):
    ins_l = [eng.lower_ap(in_)]
    for arg in (0.0, 1.0, 0.0):
        ins_l.append(mybir.ImmediateValue(dtype=mybir.dt.float32, value=arg))
    return eng.add_instruction(
        mybir.InstActivation(
            name=eng.bass.get_next_instruction_name(),
            func=ACTF.Reciprocal,
            ins=ins_l,
            outs=[eng.lower_ap(out)],
        )
    )


def _ktiles(m):
    return [m] if m == 3 else [m, m + 1]


def _kernel_body(ctx: ExitStack, tc: tile.TileContext, xy_d, wv_d, acc_d):
    nc = tc.nc

    singles = ctx.enter_context(tc.tile_pool(name="singles", bufs=1))
    xy_pool = ctx.enter_context(tc.tile_pool(name="xy", bufs=2))
    sd_pool = ctx.enter_context(tc.tile_pool(name="sd", bufs=SD_BUFS))
    h_pool = ctx.enter_context(tc.tile_pool(name="h", bufs=2))
    ch_pool = ctx.enter_context(tc.tile_pool(name="ch", bufs=3))
    ch2_pool = ctx.enter_context(tc.tile_pool(name="ch2", bufs=2))
    psum_pool = ctx.enter_context(tc.tile_pool(name="ps", bufs=2, space="PSUM"))

    wv_sb = singles.tile([128, NCLS * NP_, 128], BF16)
    acc_sb = singles.tile([128, 16], F32)
    nc.vector.memset(acc_sb[:], 0.0)

    def emit_load(p):
        xyb = xy_pool.tile([128, 2, NSEG, SEG], BF16, tag="xyb")
        nc.gpsimd.memset(xyb[:, :, :, 0:WIN], 0.0)
        e2 = nc.scalar if (LOAD_SPLIT or (LOAD_SPLIT0 and p == 0)) else nc.sync
        if P0_HALF and p == 0:
            xr = xy_d[p].rearrange("j (s q) w -> q j s w", q=128)
            for h, eng in ((0, nc.sync), (1, e2)):
                for j in (0, 1):
                    eng.dma_start(
                        out=xyb[:, j, 2 * h: 2 * h + 2, WIN:SEG],
                        in_=xr[:, j, 2 * h: 2 * h + 2, :],
                    )
            return xyb
        for j, eng in ((0, nc.sync), (1, e2)):
            eng.dma_start(
                out=xyb[:, j, :, WIN:SEG],
                in_=xy_d[p, j].rearrange("(s q) w -> q s w", q=128),
            )
        return xyb

    def emit_products(p, xyb):
        # sm = x+y, dm = x-y over the full padded buffer (pads stay 0)
        sd = sd_pool.tile([128, 2, NSEG, SEG], BF16, tag="sd")
        xf = xyb[:].rearrange("q j s c -> q j (s c)")
        sf = sd[:].rearrange("q j s c -> q j (s c)")
        for j, op, cut in ((0, ALU.add, SM_DVE[p]), (1, ALU.subtract, DM_DVE[p])):
            if cut > 0:
                nc.vector.tensor_tensor(
                    out=sf[:, j, 0:cut], in0=xf[:, 0, 0:cut],
                    in1=xf[:, 1, 0:cut], op=op)
            if cut < BUF:
                nc.gpsimd.tensor_tensor(
                    out=sf[:, j, cut:BUF], in0=xf[:, 0, cut:BUF],
                    in1=xf[:, 1, cut:BUF], op=op)
        return sd

    def emit_prodscan_halves(p, xyb):
        # fill-path: per-half products + per-map-half scans so downstream
        # matmuls (which read segment ranges) start as early as possible
        sd = sd_pool.tile([128, 2, NSEG, SEG], BF16, tag="sd")
        hsd = h_pool.tile([128, 2, BUF], BF16, tag="hsd")
        hpr = h_pool.tile([128, 2, BUF], BF16, tag="hpr")
        HB = 2 * SEG  # 1046
        xf = xyb[:].rearrange("q j s c -> q j (s c)")
        sf = sd[:].rearrange("q j s c -> q j (s c)")
        for h in (0, 1):
            lo, hi = h * HB, (h + 1) * HB
            for j, op in ((0, ALU.add), (1, ALU.subtract)):
                nc.vector.tensor_tensor(
                    out=sf[:, j, lo:hi], in0=xf[:, 0, lo:hi],
                    in1=xf[:, 1, lo:hi], op=op)
            for j in (0, 1):
                nc.vector.tensor_tensor_scan(
                    out=hsd[:, j, lo + WIN:hi],
                    data0=sf[:, j, lo + WIN:hi],
                    data1=sf[:, j, lo:hi - WIN],
                    initial=0.0, op0=ALU.add, op1=ALU.subtract)
                nc.vector._custom_dve(
                    SQSCAN,
                    out=hpr[:, j, lo + WIN:hi],
                    in0=sf[:, j, lo + WIN:hi],
                    in1=sf[:, j, lo:hi - WIN])
        return sd, (hsd, hpr)

    def emit_scans(p, sd):
        hsd = h_pool.tile([128, 2, BUF], BF16, tag="hsd")
        hpr = h_pool.tile([128, 2, BUF], BF16, tag="hpr")
        sflat = sd[:].rearrange("q j s c -> q (j s c)")
        N2 = 2 * BUF
        nc.vector.tensor_tensor_scan(
            out=hsd[:].rearrange("q a b -> q (a b)")[:, WIN:N2],
            data0=sflat[:, WIN:N2],
            data1=sflat[:, 0:N2 - WIN],
            initial=0.0, op0=ALU.add, op1=ALU.subtract,
        )
        nc.vector._custom_dve(
            SQSCAN,
            out=hpr[:].rearrange("q a b -> q (a b)")[:, WIN:N2],
            in0=sflat[:, WIN:N2],
            in1=sflat[:, 0:N2 - WIN],
        )
        return hsd, hpr

    def emit_mm(p, hsd, hpr):
        TM = ch_pool.tile([128, 2, W4], BF16, tag="TM")
        CTV = ch_pool.tile([128, 2, W4], BF16, tag="CTV")
        co = 2 * (WIN - 1) + 1  # 21
        for m in range(4):
            sl = slice(OUT * m, OUT * (m + 1))
            pq = psum_pool.tile([128, 4, 512], F32, tag="pq")
            ks = _ktiles(m)
            # PA (bank0) from hs, PB (bank1) from hd: class 0
            for bi, j in ((0, 0), (1, 1)):
                for i, k in enumerate(ks):
                    nc.tensor.matmul(
                        pq[:, bi, 0:OUT],
                        wv_sb[:, 0 * NP_ + _WIDX[(m, k)], :],
                        hsd[:, j, SEG * k + co: SEG * k + SEG],
                        start=(i == 0), stop=(i == len(ks) - 1),
                    )
            # T (bank2) = wb*hss - wb*hdd ; V2 (bank3) = wb*hss + wb*hdd
            for bi, cjs in ((2, ((1, 0), (2, 1))), (3, ((1, 0), (1, 1)))):
                nmm = len(cjs) * len(ks)
                i = 0
                for cls, j in cjs:
                    for k in ks:
                        nc.tensor.matmul(
                            pq[:, bi, 0:OUT],
                            wv_sb[:, cls * NP_ + _WIDX[(m, k)], :],
                            hpr[:, j, SEG * k + co: SEG * k + SEG],
                            start=(i == 0), stop=(i == nmm - 1),
                        )
                        i += 1
            nc.scalar.activation(
                out=TM[:, :, sl], in_=pq[:, 0:2, 0:OUT], func=ACTF.Square)
            nc.scalar.activation(
                out=CTV[:, :, sl], in_=pq[:, 2:4, 0:OUT], func=ACTF.Copy,
                bias=float(2.0 * C2H))
            if p >= NPLANE - TAIL_CHUNK:
                emit_tail_chunk(p, m, sl, TM, CTV)
        return TM, CTV

    def emit_tail_chunk(p, m, sl, TM, CTV):
        # last plane: chain per mblock, overlapping later mblocks' matmuls
        mp = OUT - 3 * 128  # 118
        np_ = 128 if m < 3 else mp
        M12 = st[p]["M12c"]
        ND = st[p]["NDc"]
        scr = st[p]["SCRc"]
        nc.vector.tensor_tensor(out=M12[:, 0, sl], in0=TM[:, 0, sl],
                                in1=TM[:, 1, sl], op=ALU.subtract)
        nc.vector.tensor_tensor(out=M12[:, 1, sl], in0=TM[:, 0, sl],
                                in1=TM[:, 1, sl], op=ALU.add)
        nc.vector._custom_dve(
            NUMDEN, out=ND[:, :, sl], in0=M12[:, :, sl], in1=CTV[:, :, sl],
            s0=float(2.0 * C1H))
        col = 16 - 4 * (NPLANE - p) + m
        nc.vector._custom_dve(
            DIVACC, out=scr[:np_, sl], accum_out=acc_sb[:np_, col: col + 1],
            in0=ND[:np_, 1, sl], in1=ND[:np_, 0, sl],
            s0=_RC["s0"], s1=_RC["s1"])

    def emit_m12(p, TM, CTV):
        M12 = ch2_pool.tile([128, 2, W4], BF16, tag="M12")
        eng1 = nc.vector if M1_ENG[p] == "v" else nc.gpsimd
        eng2 = nc.vector if M2_ENG[p] == "v" else nc.gpsimd
        eng1.tensor_tensor(out=M12[:, 0, :], in0=TM[:, 0, :],
                           in1=TM[:, 1, :], op=ALU.subtract)
        eng2.tensor_tensor(out=M12[:, 1, :], in0=TM[:, 0, :],
                           in1=TM[:, 1, :], op=ALU.add)
        if DEN_STOCK[p]:
            q1 = ch2_pool.tile([128, W4], BF16, tag="q1")
            nc.vector.tensor_scalar(out=q1[:], in0=M12[:, 1, :],
                                    scalar1=float(2.0 * C1H), scalar2=None,
                                    op0=ALU.add)
            return M12, q1
        return M12, None

    def emit_chain(p, M12, q1, CTV):
        ND = ch2_pool.tile([128, 2, W4], BF16, tag="ND")
        if DEN_STOCK[p]:
            dn = ch2_pool.tile([128, W4], BF16, tag="dn")
            nc.gpsimd.tensor_tensor(out=dn[:], in0=CTV[:, 1, :],
                                    in1=M12[:, 1, :], op=ALU.subtract)
            nc.gpsimd.tensor_tensor(out=ND[:, 1, :], in0=q1[:], in1=dn[:],
                                    op=ALU.mult)
            nc.vector._custom_dve(
                NUMDEN, out=ND[:, 0, :], in0=M12[:, 0, :],
                in1=CTV[:, 0, :], s0=float(2.0 * C1H),
            )
        else:
            nc.vector._custom_dve(
                NUMDEN,
                out=ND[:].rearrange("q a b -> q (a b)"),
                in0=M12[:].rearrange("q a b -> q (a b)"),
                in1=CTV[:].rearrange("q a b -> q (a b)"),
                s0=float(2.0 * C1H),
            )
        return ND

    def emit_div(p, ND, TM):
        c0 = 3 * OUT  # 1506
        mp = OUT - 3 * 128  # 118
        scr_t = ch2_pool.tile([128, W4], BF16, tag="scr")
        scr = scr_t[:]
        if TAIL_STOCK[p]:
            rcp_t = ch2_pool.tile([128, W4], BF16, tag="rcp")
            _act_recip(nc.scalar, rcp_t[:], ND[:, 1, :])
            nc.vector.tensor_tensor(out=scr[:, 0:c0], in0=ND[:, 0, 0:c0],
                                    in1=rcp_t[:, 0:c0], op=ALU.mult)
            nc.vector.tensor_tensor(out=scr[:mp, c0:W4],
                                    in0=ND[:mp, 0, c0:W4],
                                    in1=rcp_t[:mp, c0:W4], op=ALU.mult)
            nc.scalar.activation(
                out=ND[:, 1, 0:c0], in_=scr[:, 0:c0], func=ACTF.Copy,
                accum_out=acc_sb[:, 2 * p: 2 * p + 1])
            nc.scalar.activation(
                out=ND[:mp, 1, c0:W4], in_=scr[:mp, c0:W4], func=ACTF.Copy,
                accum_out=acc_sb[:mp, 2 * p + 1: 2 * p + 2])
            return
        nc.vector._custom_dve(
            DIVACC, out=scr[:, 0:c0], accum_out=acc_sb[:, 2 * p: 2 * p + 1],
            in0=ND[:, 1, 0:c0], in1=ND[:, 0, 0:c0],
            s0=_RC["s0"], s1=_RC["s1"],
        )
        nc.vector._custom_dve(
            DIVACC, out=scr[:mp, c0:W4],
            accum_out=acc_sb[:mp, 2 * p + 1: 2 * p + 2],
            in0=ND[:mp, 1, c0:W4], in1=ND[:mp, 0, c0:W4],
            s0=_RC["s0"], s1=_RC["s1"],
        )

    # ---- software pipeline ----
    s1_it = [p + 1 for p in range(NPLANE)]  # products + scans
    s2_it = [p + 2 for p in range(NPLANE)]  # mm + act pairs
    s3a_it = [p + 3 for p in range(NPLANE)]  # m1/m2
    s3b_it = [p + 4 for p in range(NPLANE)]  # numden + div
    s3b_it[NPLANE - 1] -= 1  # compress last plane's tail
    st = {p: {} for p in range(NPLANE)}
    for it in range(max(s3b_it) + 1):
        # ready work first: loads, then products+scans (deps one it old),
        # then mm, then the cross-engine chain stages.
        if it < NPLANE:
            st[it]["xyb"] = emit_load(it)
        if it == 0:
            nc.sync.dma_start(out=wv_sb[:], in_=wv_d)
        if POOL_M12_FIRST:
            for p in range(NPLANE):
                if s3a_it[p] == it and s3b_it[p] != it:
                    st[p]["M12"] = emit_m12(p, *st[p]["mm"])
        if P0_HALF and it == 1 and "h" not in st[0]:
            st[0]["sd"], st[0]["h"] = emit_prodscan_halves(0, st[0]["xyb"])
        if PROD_EARLY == 1 and it < NPLANE:
            st[it]["sd"] = emit_products(it, st[it]["xyb"])
        for p in range(NPLANE):
            if s1_it[p] == it:
                if P0_HALF and p == 0:
                    continue
                if not PROD_EARLY:
                    st[p]["sd"] = emit_products(p, st[p]["xyb"])
                st[p]["h"] = emit_scans(p, st[p]["sd"])
        if not POOL_M12_FIRST:
            for p in range(NPLANE):
                if s3a_it[p] == it and s3b_it[p] != it:
                    st[p]["M12"] = emit_m12(p, *st[p]["mm"])
        for p in range(NPLANE):
            if s3b_it[p] == it and s3a_it[p] != it:
                st[p]["ND"] = emit_chain(p, *st[p]["M12"], st[p]["mm"][1])
                emit_div(p, st[p]["ND"], st[p]["mm"][0])
        for p in range(NPLANE):
            if s2_it[p] == it:
                if p >= NPLANE - TAIL_CHUNK:
                    m12c = ch2_pool.tile([128, 2, W4], BF16, tag="M12")
                    ndc = ch2_pool.tile([128, 2, W4], BF16, tag="ND")
                    scrc = ch2_pool.tile([128, W4], BF16, tag="scr")
                    st[p]["M12c"] = m12c
                    st[p]["NDc"] = ndc
                    st[p]["SCRc"] = scrc
                st[p]["mm"] = emit_mm(p, *st[p]["h"])
        for p in range(NPLANE):
            if s3a_it[p] == it and s3b_it[p] == it:
                if p >= NPLANE - TAIL_CHUNK:
                    continue
                st[p]["M12"] = emit_m12(p, *st[p]["mm"])
                st[p]["ND"] = emit_chain(p, *st[p]["M12"], st[p]["mm"][1])
                emit_div(p, st[p]["ND"], st[p]["mm"][0])
        if PROD_EARLY == 2 and it < NPLANE and not (P0_HALF and it == 0):
            st[it]["sd"] = emit_products(it, st[it]["xyb"])

    nc.sync.dma_start(out=acc_d, in_=acc_sb[:])


_CACHE = {}


class _Bacc(bacc.Bacc):
    def insert_act_table_loads(self):
        import bass_rust as _br
        from concourse.hw_specs import get_activation_tables

        tables = [
            (n, (f if n == "reciprocal_and_small" else set()))
            for n, f in get_activation_tables(self.m.arch).items()
        ]
        _br.insert_act_table_loads(self, tables)


def _get_nc():
    if "nc" in _CACHE:
        return _CACHE["nc"]
    nc = _Bacc("TRN2", target_bir_lowering=False, debug=False)
    xy_d = nc.dram_tensor(
        "xy", [NPLANE, 2, IMG, IMG], BF16, kind="ExternalInput").ap()
    wv_d = nc.dram_tensor(
        "wv", [128, NCLS * NP_, 128], BF16, kind="ExternalInput").ap()
    acc_d = nc.dram_tensor("acc", [128, 16], F32, kind="ExternalOutput").ap()
    with tile.TileContext(nc) as tc, ExitStack() as ctx:
        _kernel_body(ctx, tc, xy_d, wv_d, acc_d)
    nc.compile()
    _CACHE["nc"] = nc
    return nc


def _run(x, y, trace=False, **kw):
    nc = _get_nc()
    wv = _build_weights()
    x = np.asarray(x, dtype=np.float32).astype(ml_dtypes.bfloat16)
    y = np.asarray(y, dtype=np.float32).astype(ml_dtypes.bfloat16)
    b_per = x.shape[0] // NCORES
    in_maps = []
    for c in range(NCORES):
        xs = x[c * b_per: (c + 1) * b_per].reshape(NPLANE, IMG, IMG)
        ys = y[c * b_per: (c + 1) * b_per].reshape(NPLANE, IMG, IMG)
        xy = np.ascontiguousarray(np.stack([xs, ys], axis=1))
        in_maps.append({"xy": xy, "wv": wv})
    res = bass_utils.run_bass_kernel_spmd(
        nc, in_maps, core_ids=list(range(NCORES)), trace=trace, **kw
    )
    total = 0.0
    for r in res.results:
        total += r["acc"].astype(np.float64).sum()
    mean = total / float(16 * 3 * OUT * OUT)
    out = np.float32(1.0 - mean)
    return out, res


def kernel(x, y):
    out, _ = _run(x, y, trace=False)
    return out


# revision 5
# speedup vs baseline: 1.0072x; 1.0072x over previous
"""SSIM loss kernel for Trainium2, v18: s/d basis + custom DVE ops.

Per core: 6 planes of 512x512. Host casts x,y to bf16 and stacks.
Per plane:
  sm=x+y, dm=x-y (TT, split DVE/Pool by column knob)
  scan1: stock merged scan over [sm,dm] -> hs,hd        (DVE)
  SQSCAN: custom scan h += Src0^2-Src1^2 over [sm,dm] -> hss,hdd (DVE)
  42 matmuls/plane: PA=w.hs, PB=w.hd, T=wb.hss-wb.hdd, V2=wb.(hss+hdd)
  Act: Square pair (PA,PB)->MS,MD ; Copy+2c2 pair (T,V2)->cT,cV
  m1=MS-MD, m2=MS+MD (TT, engine knobs)
  NUMDEN custom: [num,den] = (M12+2c1)*(CTV-M12) paired   (DVE)
  DIVACC custom: ssim=num*recip_nr1(den), accum -> acc    (DVE)
Host: 1 - sum(acc)/(48*502*502).
"""

import sys
from contextlib import ExitStack

import numpy as np

sys.path.insert(0, "/opt/trn_rl_repo")

import ml_dtypes  # noqa: E402

import concourse.bass as bass  # noqa: E402
import concourse.tile as tile  # noqa: E402
from concourse import bacc, bass_utils, mybir  # noqa: E402
from concourse import dve_ops  # noqa: E402
from concourse.dve_spec import (  # noqa: E402
    AluOp, Bin, C0, C1, Spec, Src0, Src1, lower, scan, sq,
)
from concourse.dve_uop import DveOpSpec  # noqa: E402

F32 = mybir.dt.float32
BF16 = mybir.dt.bfloat16
ALU = mybir.AluOpType
ACTF = mybir.ActivationFunctionType

WIN = 11
IMG = 512
OUT = IMG - WIN + 1  # 502
SEG = WIN + IMG  # 523
NSEG = 4
BUF = NSEG * SEG  # 2092
NPLANE = 6
NCORES = 8
W4 = 4 * OUT  # 2008

C1C = (0.01 * 1.0) ** 2
C2C = (0.03 * 1.0) ** 2
G = 121.0 / 128.0
C1H = np.float32(C1C * G * G)
C2H = np.float32(C2C * G * G)
W_A = float(2.0 ** -7)
W_B = float(121.0 * 2.0 ** -14)
NCLS = 3

_PAIRS = [(0, 0), (0, 1), (1, 1), (1, 2), (2, 2), (2, 3), (3, 3)]
_WIDX = {mk: i for i, mk in enumerate(_PAIRS)}
NP_ = len(_PAIRS)

# ---- knobs ---------------------------------------------------------------
import os as _os


def _env(name, default):
    v = _os.environ.get("V18_" + name)
    if v is None:
        return default
    return eval(v)  # noqa: S307 - trusted local tuning knob


# per-plane: first *_DVE cols of sm/dm on DVE, rest on Pool
SM_DVE = _env("SM_DVE", (BUF, 1046, 1046, 1046, 1046, 1046))
DM_DVE = _env("DM_DVE", (BUF, 0, 0, 0, 0, 0))
# products emitted in the load iteration (Pool gets a stage of slack)
PROD_EARLY = _env("PROD_EARLY", 2)
# per-plane m1/m2 engine: v=DVE, p=Pool (last plane on DVE to cut drain)
M1_ENG = _env("M1_ENG", ("p", "p", "p", "p", "p", "v"))
M2_ENG = _env("M2_ENG", ("p", "p", "p", "p", "p", "v"))
# per-plane: den lane via stock ops on Pool/Act instead of fused NUMDEN
DEN_STOCK = _env("DEN_STOCK", (False,) * 6)
LOAD_SPLIT = _env("LOAD_SPLIT", 0)  # 1: y-load on Act DMA ring
SD_BUFS = _env("SD_BUFS", 3)
# per-plane: tail via Act reciprocal + DVE 2x TT + Act accum (else DIVACC)
TAIL_STOCK = _env("TAIL_STOCK", (True, True, True, True, True, False))
POOL_M12_FIRST = _env("POOL_M12_FIRST", 0)
TAIL_CHUNK = _env("TAIL_CHUNK", 1)  # last plane: per-mblock chain after its act pairs
LOAD_SPLIT0 = _env("LOAD_SPLIT0", 1)  # plane 0 loads on two DMA rings
P0_HALF = _env("P0_HALF", 1)  # plane 0: half-plane loads/products/scans
ND_SPLIT = _env("ND_SPLIT", 1)  # split NUMDEN into num-op + den-op


# ---- custom DVE ops ------------------------------------------------------
def _register(name, spec, subdim=False):
    for op in dve_ops.OPS:
        if op.name == name:
            return op
    shas = {}
    for ver in ("v3", "v4"):
        s = DveOpSpec(name=name, opcode=0, uops=lower(spec, ver=ver),
                      rd1_en=True)
        shas[ver] = s.sha(ver)
    op = dve_ops.DveOp(name, spec, subdim=subdim, uops_sha=shas)
    dve_ops.OPS.append(op)
    dve_ops.CUSTOM_DVE_SPECS[name] = spec
    dve_ops._SUB_OPCODE_FOR_NAME[name] = (
        dve_ops._CUSTOM_DVE_ROW_BASE + len(dve_ops.OPS) - 1
    )
    return op


SQSCAN = _register(
    "SSIM_SQSCAN",
    Spec(
        body=scan(AluOp.ADD, sq(Src0) - sq(Src1)),
        reference=lambda in0, in1, c0, c1, c2: np.cumsum(
            in0.astype(np.float32) ** 2 - in1.astype(np.float32) ** 2,
            axis=-1,
        ),
    ),
)

NUMDEN = _register(
    "SSIM_NUMDEN",
    Spec(
        body=(Src0 + C0) * (Src1 - Src0),
        reference=lambda in0, in1, c0, c1, c2: (
            (in0.astype(np.float32) + c0) * (in1 - in0)
        ),
    ),
)

_not = Bin(AluOp.BITWISE_NOT, Src0, Src0)
_y0 = _not * C0
_y1 = _y0 * (C1 - Src0 * _y0)


def _ref_divacc(in0, in1, c0, c1, c2):
    nx = (~in0.astype(np.float32).view(np.int32)).view(np.float32)
    y0 = nx * c0
    y1 = y0 * (c1 - in0 * y0)
    o = (in1 * y1).astype(np.float32)
    return o, o.reshape(o.shape[0], -1).sum(-1, keepdims=True)


DIVACC = _register(
    "SSIM_DIVACC",
    Spec(body=Src1 * _y1, accum=AluOp.ADD, reference=_ref_divacc),
)
_RC = dve_ops.RECIP_APPROX_FAST_CONSTS


def _build_weights() -> np.ndarray:
    w = np.zeros((NCLS, NP_, 128, 128), dtype=np.float32)
    vals = [W_A, W_B, -W_B]
    for idx, (m, k) in enumerate(_PAIRS):
        for i in range(128):
            for o in range(128):
                d = (128 * k + i) - (128 * m + o)
                if 0 <= d < WIN:
                    for c in range(NCLS):
                        w[c, idx, i, o] = vals[c]
    return np.ascontiguousarray(
        w.transpose(2, 0, 1, 3).reshape(128, NCLS * NP_, 128)
    ).astype(ml_dtypes.bfloat16)


def _act_recip(eng, out, in_):
    ins_l = [eng.lower_ap(in_)]
    for arg in (0.0, 1.0, 0.0):
        ins_l.append(mybir.ImmediateValue(dtype=mybir.dt.float32, value=arg))
    return eng.add_instruction(
        mybir.InstActivation(
            name=eng.bass.get_next_instruction_name(),
            func=ACTF.Reciprocal,
            ins=ins_l,
            outs=[eng.lower_ap(out)],
        )
    )


def _ktiles(m):
    return [m] if m == 3 else [m, m + 1]


def _kernel_body(ctx: ExitStack, tc: tile.TileContext, xy_d, wv_d, acc_d):
    nc = tc.nc

    singles = ctx.enter_context(tc.tile_pool(name="singles", bufs=1))
    xy_pool = ctx.enter_context(tc.tile_pool(name="xy", bufs=2))
    sd_pool = ctx.enter_context(tc.tile_pool(name="sd", bufs=SD_BUFS))
    h_pool = ctx.enter_context(tc.tile_pool(name="h", bufs=2))
    ch_pool = ctx.enter_context(tc.tile_pool(name="ch", bufs=3))
    ch2_pool = ctx.enter_context(tc.tile_pool(name="ch2", bufs=2))
    psum_pool = ctx.enter_context(tc.tile_pool(name="ps", bufs=2, space="PSUM"))

    wv_sb = singles.tile([128, NCLS * NP_, 128], BF16)
    acc_sb = singles.tile([128, 16], F32)
    nc.vector.memset(acc_sb[:], 0.0)

    def emit_load(p):
        xyb = xy_pool.tile([128, 2, NSEG, SEG], BF16, tag="xyb")
        nc.gpsimd.memset(xyb[:, :, :, 0:WIN], 0.0)
        e2 = nc.scalar if (LOAD_SPLIT or (LOAD_SPLIT0 and p == 0)) else nc.sync
        if P0_HALF and p == 0:
            xr = xy_d[p].rearrange("j (s q) w -> q j s w", q=128)
            for h, eng in ((0, nc.sync), (1, e2)):
                for j in (0, 1):
                    eng.dma_start(
                        out=xyb[:, j, 2 * h: 2 * h + 2, WIN:SEG],
                        in_=xr[:, j, 2 * h: 2 * h + 2, :],
                    )
            return xyb
        for j, eng in ((0, nc.sync), (1, e2)):
            eng.dma_start(
                out=xyb[:, j, :, WIN:SEG],
                in_=xy_d[p, j].rearrange("(s q) w -> q s w", q=128),
            )
        return xyb

    def emit_products(p, xyb):
        # sm = x+y, dm = x-y over the full padded buffer (pads stay 0)
        sd = sd_pool.tile([128, 2, NSEG, SEG], BF16, tag="sd")
        xf = xyb[:].rearrange("q j s c -> q j (s c)")
        sf = sd[:].rearrange("q j s c -> q j (s c)")
        for j, op, cut in ((0, ALU.add, SM_DVE[p]), (1, ALU.subtract, DM_DVE[p])):
            if cut > 0:
                nc.vector.tensor_tensor(
                    out=sf[:, j, 0:cut], in0=xf[:, 0, 0:cut],
                    in1=xf[:, 1, 0:cut], op=op)
            if cut < BUF:
                nc.gpsimd.tensor_tensor(
                    out=sf[:, j, cut:BUF], in0=xf[:, 0, cut:BUF],
                    in1=xf[:, 1, cut:BUF], op=op)
        return sd

    def emit_prodscan_halves(p, xyb):
        # fill-path: per-half products + per-map-half scans so downstream
        # matmuls (which read segment ranges) start as early as possible
        sd = sd_pool.tile([128, 2, NSEG, SEG], BF16, tag="sd")
        hsd = h_pool.tile([128, 2, BUF], BF16, tag="hsd")
        hpr = h_pool.tile([128, 2, BUF], BF16, tag="hpr")
        HB = 2 * SEG  # 1046
        xf = xyb[:].rearrange("q j s c -> q j (s c)")
        sf = sd[:].rearrange("q j s c -> q j (s c)")
        for h in (0, 1):
            lo, hi = h * HB, (h + 1) * HB
            for j, op in ((0, ALU.add), (1, ALU.subtract)):
                nc.vector.tensor_tensor(
                    out=sf[:, j, lo:hi], in0=xf[:, 0, lo:hi],
                    in1=xf[:, 1, lo:hi], op=op)
            for j in (0, 1):
                nc.vector.tensor_tensor_scan(
                    out=hsd[:, j, lo + WIN:hi],
                    data0=sf[:, j, lo + WIN:hi],
                    data1=sf[:, j, lo:hi - WIN],
                    initial=0.0, op0=ALU.add, op1=ALU.subtract)
                nc.vector._custom_dve(
                    SQSCAN,
                    out=hpr[:, j, lo + WIN:hi],
                    in0=sf[:, j, lo + WIN:hi],
                    in1=sf[:, j, lo:hi - WIN])
        return sd, (hsd, hpr)

    def emit_scans(p, sd):
        hsd = h_pool.tile([128, 2, BUF], BF16, tag="hsd")
        hpr = h_pool.tile([128, 2, BUF], BF16, tag="hpr")
        sflat = sd[:].rearrange("q j s c -> q (j s c)")
        N2 = 2 * BUF
        nc.vector.tensor_tensor_scan(
            out=hsd[:].rearrange("q a b -> q (a b)")[:, WIN:N2],
            data0=sflat[:, WIN:N2],
            data1=sflat[:, 0:N2 - WIN],
            initial=0.0, op0=ALU.add, op1=ALU.subtract,
        )
        nc.vector._custom_dve(
            SQSCAN,
            out=hpr[:].rearrange("q a b -> q (a b)")[:, WIN:N2],
            in0=sflat[:, WIN:N2],
            in1=sflat[:, 0:N2 - WIN],
        )
        return hsd, hpr

    def emit_mm(p, hsd, hpr):
        TM = ch_pool.tile([128, 2, W4], BF16, tag="TM")
        CTV = ch_pool.tile([128, 2, W4], BF16, tag="CTV")
        co = 2 * (WIN - 1) + 1  # 21
        for m in range(4):
            sl = slice(OUT * m, OUT * (m + 1))
            pq = psum_pool.tile([128, 4, 512], F32, tag="pq")
            ks = _ktiles(m)
            # PA (bank0) from hs, PB (bank1) from hd: class 0
            for bi, j in ((0, 0), (1, 1)):
                for i, k in enumerate(ks):
                    nc.tensor.matmul(
                        pq[:, bi, 0:OUT],
                        wv_sb[:, 0 * NP_ + _WIDX[(m, k)], :],
                        hsd[:, j, SEG * k + co: SEG * k + SEG],
                        start=(i == 0), stop=(i == len(ks) - 1),
                    )
            # T (bank2) = wb*hss - wb*hdd ; V2 (bank3) = wb*hss + wb*hdd
            for bi, cjs in ((2, ((1, 0), (2, 1))), (3, ((1, 0), (1, 1)))):
                nmm = len(cjs) * len(ks)
                i = 0
                for cls, j in cjs:
                    for k in ks:
                        nc.tensor.matmul(
                            pq[:, bi, 0:OUT],
                            wv_sb[:, cls * NP_ + _WIDX[(m, k)], :],
                            hpr[:, j, SEG * k + co: SEG * k + SEG],
                            start=(i == 0), stop=(i == nmm - 1),
                        )
                        i += 1
            nc.scalar.activation(
                out=TM[:, :, sl], in_=pq[:, 0:2, 0:OUT], func=ACTF.Square)
            nc.scalar.activation(
                out=CTV[:, :, sl], in_=pq[:, 2:4, 0:OUT], func=ACTF.Copy,
                bias=float(2.0 * C2H))
            if p >= NPLANE - TAIL_CHUNK:
                emit_tail_chunk(p, m, sl, TM, CTV)
        return TM, CTV

    def emit_tail_chunk(p, m, sl, TM, CTV):
        # last plane: chain per mblock, overlapping later mblocks' matmuls
        mp = OUT - 3 * 128  # 118
        np_ = 128 if m < 3 else mp
        M12 = st[p]["M12c"]
        ND = st[p]["NDc"]
        scr = st[p]["SCRc"]
        nc.vector.tensor_tensor(out=M12[:, 0, sl], in0=TM[:, 0, sl],
                                in1=TM[:, 1, sl], op=ALU.subtract)
        nc.vector.tensor_tensor(out=M12[:, 1, sl], in0=TM[:, 0, sl],
                                in1=TM[:, 1, sl], op=ALU.add)
        nc.vector._custom_dve(
            NUMDEN, out=ND[:, :, sl], in0=M12[:, :, sl], in1=CTV[:, :, sl],
            s0=float(2.0 * C1H))
        col = 16 - 4 * (NPLANE - p) + m
        nc.vector._custom_dve(
            DIVACC, out=scr[:np_, sl], accum_out=acc_sb[:np_, col: col + 1],
            in0=ND[:np_, 1, sl], in1=ND[:np_, 0, sl],
            s0=_RC["s0"], s1=_RC["s1"])

    def emit_m12(p, TM, CTV):
        M12 = ch2_pool.tile([128, 2, W4], BF16, tag="M12")
        eng1 = nc.vector if M1_ENG[p] == "v" else nc.gpsimd
        eng2 = nc.vector if M2_ENG[p] == "v" else nc.gpsimd
        eng1.tensor_tensor(out=M12[:, 0, :], in0=TM[:, 0, :],
                           in1=TM[:, 1, :], op=ALU.subtract)
        eng2.tensor_tensor(out=M12[:, 1, :], in0=TM[:, 0, :],
                           in1=TM[:, 1, :], op=ALU.add)
        if DEN_STOCK[p]:
            q1 = ch2_pool.tile([128, W4], BF16, tag="q1")
            nc.vector.tensor_scalar(out=q1[:], in0=M12[:, 1, :],
                                    scalar1=float(2.0 * C1H), scalar2=None,
                                    op0=ALU.add)
            return M12, q1
        return M12, None

    def emit_chain(p, M12, q1, CTV):
        ND = ch2_pool.tile([128, 2, W4], BF16, tag="ND")
        if DEN_STOCK[p]:
            dn = ch2_pool.tile([128, W4], BF16, tag="dn")
            nc.gpsimd.tensor_tensor(out=dn[:], in0=CTV[:, 1, :],
                                    in1=M12[:, 1, :], op=ALU.subtract)
            nc.gpsimd.tensor_tensor(out=ND[:, 1, :], in0=q1[:], in1=dn[:],
                                    op=ALU.mult)
            nc.vector._custom_dve(
                NUMDEN, out=ND[:, 0, :], in0=M12[:, 0, :],
                in1=CTV[:, 0, :], s0=float(2.0 * C1H),
            )
        elif ND_SPLIT:
            nc.vector._custom_dve(
                NUMDEN, out=ND[:, 0, :], in0=M12[:, 0, :],
                in1=CTV[:, 0, :], s0=float(2.0 * C1H))
            nc.vector._custom_dve(
                NUMDEN, out=ND[:, 1, :], in0=M12[:, 1, :],
                in1=CTV[:, 1, :], s0=float(2.0 * C1H))
        else:
            nc.vector._custom_dve(
                NUMDEN,
                out=ND[:].rearrange("q a b -> q (a b)"),
                in0=M12[:].rearrange("q a b -> q (a b)"),
                in1=CTV[:].rearrange("q a b -> q (a b)"),
                s0=float(2.0 * C1H),
            )
        return ND

    def emit_div(p, ND, TM):
        c0 = 3 * OUT  # 1506
        mp = OUT - 3 * 128  # 118
        scr_t = ch2_pool.tile([128, W4], BF16, tag="scr")
        scr = scr_t[:]
        if TAIL_STOCK[p]:
            rcp_t = ch2_pool.tile([128, W4], BF16, tag="rcp")
            _act_recip(nc.scalar, rcp_t[:], ND[:, 1, :])
            nc.vector.tensor_tensor(out=scr[:, 0:c0], in0=ND[:, 0, 0:c0],
                                    in1=rcp_t[:, 0:c0], op=ALU.mult)
            nc.vector.tensor_tensor(out=scr[:mp, c0:W4],
                                    in0=ND[:mp, 0, c0:W4],
                                    in1=rcp_t[:mp, c0:W4], op=ALU.mult)
            nc.scalar.activation(
                out=ND[:, 1, 0:c0], in_=scr[:, 0:c0], func=ACTF.Copy,
                accum_out=acc_sb[:, 2 * p: 2 * p + 1])
            nc.scalar.activation(
                out=ND[:mp, 1, c0:W4], in_=scr[:mp, c0:W4], func=ACTF.Copy,
                accum_out=acc_sb[:mp, 2 * p + 1: 2 * p + 2])
            return
        nc.vector._custom_dve(
            DIVACC, out=scr[:, 0:c0], accum_out=acc_sb[:, 2 * p: 2 * p + 1],
            in0=ND[:, 1, 0:c0], in1=ND[:, 0, 0:c0],
            s0=_RC["s0"], s1=_RC["s1"],
        )
        nc.vector._custom_dve(
            DIVACC, out=scr[:mp, c0:W4],
            accum_out=acc_sb[:mp, 2 * p + 1: 2 * p + 2],
            in0=ND[:mp, 1, c0:W4], in1=ND[:mp, 0, c0:W4],
            s0=_RC["s0"], s1=_RC["s1"],
        )

    # ---- software pipeline ----
    s1_it = [p + 1 for p in range(NPLANE)]  # products + scans
    s2_it = [p + 2 for p in range(NPLANE)]  # mm + act pairs
    s3a_it = [p + 3 for p in range(NPLANE)]  # m1/m2
    s3b_it = [p + 4 for p in range(NPLANE)]  # numden + div
    s3b_it[NPLANE - 1] -= 1  # compress last plane's tail
    st = {p: {} for p in range(NPLANE)}
    for it in range(max(s3b_it) + 1):
        # ready work first: loads, then products+scans (deps one it old),
        # then mm, then the cross-engine chain stages.
        if it < NPLANE:
            st[it]["xyb"] = emit_load(it)
        if it == 0:
            nc.sync.dma_start(out=wv_sb[:], in_=wv_d)
        if POOL_M12_FIRST:
            for p in range(NPLANE):
                if s3a_it[p] == it and s3b_it[p] != it:
                    st[p]["M12"] = emit_m12(p, *st[p]["mm"])
        if P0_HALF and it == 1 and "h" not in st[0]:
            st[0]["sd"], st[0]["h"] = emit_prodscan_halves(0, st[0]["xyb"])
        if PROD_EARLY == 1 and it < NPLANE:
            st[it]["sd"] = emit_products(it, st[it]["xyb"])
        for p in range(NPLANE):
            if s1_it[p] == it:
                if P0_HALF and p == 0:
                    continue
                if not PROD_EARLY:
                    st[p]["sd"] = emit_products(p, st[p]["xyb"])
                st[p]["h"] = emit_scans(p, st[p]["sd"])
        if not POOL_M12_FIRST:
            for p in range(NPLANE):
                if s3a_it[p] == it and s3b_it[p] != it:
                    st[p]["M12"] = emit_m12(p, *st[p]["mm"])
        for p in range(NPLANE):
            if s3b_it[p] == it and s3a_it[p] != it:
                st[p]["ND"] = emit_chain(p, *st[p]["M12"], st[p]["mm"][1])
                emit_div(p, st[p]["ND"], st[p]["mm"][0])
        for p in range(NPLANE):
            if s2_it[p] == it:
                if p >= NPLANE - TAIL_CHUNK:
                    m12c = ch2_pool.tile([128, 2, W4], BF16, tag="M12")
                    ndc = ch2_pool.tile([128, 2, W4], BF16, tag="ND")
                    scrc = ch2_pool.tile([128, W4], BF16, tag="scr")
                    st[p]["M12c"] = m12c
                    st[p]["NDc"] = ndc
                    st[p]["SCRc"] = scrc
                st[p]["mm"] = emit_mm(p, *st[p]["h"])
        for p in range(NPLANE):
            if s3a_it[p] == it and s3b_it[p] == it:
                if p >= NPLANE - TAIL_CHUNK:
                    continue
                st[p]["M12"] = emit_m12(p, *st[p]["mm"])
                st[p]["ND"] = emit_chain(p, *st[p]["M12"], st[p]["mm"][1])
                emit_div(p, st[p]["ND"], st[p]["mm"][0])
        if PROD_EARLY == 2 and it < NPLANE and not (P0_HALF and it == 0):
            st[it]["sd"] = emit_products(it, st[it]["xyb"])

    nc.sync.dma_start(out=acc_d, in_=acc_sb[:])


_CACHE = {}


class _Bacc(bacc.Bacc):
    def insert_act_table_loads(self):
        import bass_rust as _br
        from concourse.hw_specs import get_activation_tables

        tables = [
            (n, (f if n == "reciprocal_and_small" else set()))
            for n, f in get_activation_tables(self.m.arch).items()
        ]
        _br.insert_act_table_loads(self, tables)


def _get_nc():
    if "nc" in _CACHE:
        return _CACHE["nc"]
    nc = _Bacc("TRN2", target_bir_lowering=False, debug=False)
    xy_d = nc.dram_tensor(
        "xy", [NPLANE, 2, IMG, IMG], BF16, kind="ExternalInput").ap()
    wv_d = nc.dram_tensor(
        "wv", [128, NCLS * NP_, 128], BF16, kind="ExternalInput").ap()
    acc_d = nc.dram_tensor("acc", [128, 16], F32, kind="ExternalOutput").ap()
    with tile.TileContext(nc) as tc, ExitStack() as ctx:
        _kernel_body(ctx, tc, xy_d, wv_d, acc_d)
    nc.compile()
    _CACHE["nc"] = nc
    return nc


def _run(x, y, trace=False, **kw):
    nc = _get_nc()
    wv = _build_weights()
    x = np.asarray(x, dtype=np.float32).astype(ml_dtypes.bfloat16)
    y = np.asarray(y, dtype=np.float32).astype(ml_dtypes.bfloat16)
    b_per = x.shape[0] // NCORES
    in_maps = []
    for c in range(NCORES):
        xs = x[c * b_per: (c + 1) * b_per].reshape(NPLANE, IMG, IMG)
        ys = y[c * b_per: (c + 1) * b_per].reshape(NPLANE, IMG, IMG)
        xy = np.ascontiguousarray(np.stack([xs, ys], axis=1))
        in_maps.append({"xy": xy, "wv": wv})
    res = bass_utils.run_bass_kernel_spmd(
        nc, in_maps, core_ids=list(range(NCORES)), trace=trace, **kw
    )
    total = 0.0
    for r in res.results:
        total += r["acc"].astype(np.float64).sum()
    mean = total / float(16 * 3 * OUT * OUT)
    out = np.float32(1.0 - mean)
    return out, res


def kernel(x, y):
    out, _ = _run(x, y, trace=False)
    return out


# revision 6
# speedup vs baseline: 1.0080x; 1.0009x over previous
"""SSIM loss kernel for Trainium2, v18: s/d basis + custom DVE ops.

Per core: 6 planes of 512x512. Host casts x,y to bf16 and stacks.
Per plane:
  sm=x+y, dm=x-y (TT, split DVE/Pool by column knob)
  scan1: stock merged scan over [sm,dm] -> hs,hd        (DVE)
  SQSCAN: custom scan h += Src0^2-Src1^2 over [sm,dm] -> hss,hdd (DVE)
  42 matmuls/plane: PA=w.hs, PB=w.hd, T=wb.hss-wb.hdd, V2=wb.(hss+hdd)
  Act: Square pair (PA,PB)->MS,MD ; Copy+2c2 pair (T,V2)->cT,cV
  m1=MS-MD, m2=MS+MD (TT, engine knobs)
  NUMDEN custom: [num,den] = (M12+2c1)*(CTV-M12) paired   (DVE)
  DIVACC custom: ssim=num*recip_nr1(den), accum -> acc    (DVE)
Host: 1 - sum(acc)/(48*502*502).
"""

import sys
from contextlib import ExitStack

import numpy as np

sys.path.insert(0, "/opt/trn_rl_repo")

import ml_dtypes  # noqa: E402

import concourse.bass as bass  # noqa: E402
import concourse.tile as tile  # noqa: E402
from concourse import bacc, bass_utils, mybir  # noqa: E402
from concourse import dve_ops  # noqa: E402
from concourse.dve_spec import (  # noqa: E402
    AluOp, Bin, C0, C1, C2, Spec, Src0, Src1, lower, scan, sq,
)
from concourse.dve_uop import DveOpSpec  # noqa: E402

F32 = mybir.dt.float32
BF16 = mybir.dt.bfloat16
ALU = mybir.AluOpType
ACTF = mybir.ActivationFunctionType

WIN = 11
IMG = 512
OUT = IMG - WIN + 1  # 502
SEG = WIN + IMG  # 523
NSEG = 4
BUF = NSEG * SEG  # 2092
NPLANE = 6
NCORES = 8
W4 = 4 * OUT  # 2008

C1C = (0.01 * 1.0) ** 2
C2C = (0.03 * 1.0) ** 2
G = 121.0 / 128.0
C1H = np.float32(C1C * G * G)
C2H = np.float32(C2C * G * G)
W_A = float(2.0 ** -7)
W_B = float(121.0 * 2.0 ** -14)
NCLS = 3

_PAIRS = [(0, 0), (0, 1), (1, 1), (1, 2), (2, 2), (2, 3), (3, 3)]
_WIDX = {mk: i for i, mk in enumerate(_PAIRS)}
NP_ = len(_PAIRS)

# ---- knobs ---------------------------------------------------------------
import os as _os


def _env(name, default):
    v = _os.environ.get("V18_" + name)
    if v is None:
        return default
    return eval(v)  # noqa: S307 - trusted local tuning knob


# per-plane: first *_DVE cols of sm/dm on DVE, rest on Pool
SM_DVE = _env("SM_DVE", (BUF, 1046, 1046, 1046, 1046, 1046))
DM_DVE = _env("DM_DVE", (BUF, 0, 0, 0, 0, 0))
# products emitted in the load iteration (Pool gets a stage of slack)
PROD_EARLY = _env("PROD_EARLY", 2)
# per-plane m1/m2 engine: v=DVE, p=Pool (last plane on DVE to cut drain)
M1_ENG = _env("M1_ENG", ("p", "p", "p", "p", "p", "v"))
M2_ENG = _env("M2_ENG", ("p", "p", "p", "p", "p", "v"))
# per-plane: den lane via stock ops on Pool/Act instead of fused NUMDEN
DEN_STOCK = _env("DEN_STOCK", (False,) * 6)
LOAD_SPLIT = _env("LOAD_SPLIT", 0)  # 1: y-load on Act DMA ring
SD_BUFS = _env("SD_BUFS", 3)
H_BUFS = _env("H_BUFS", 2)
CH_BUFS = _env("CH_BUFS", 3)
CH2_BUFS = _env("CH2_BUFS", 2)
# per-plane: tail via Act reciprocal + DVE 2x TT + Act accum (else DIVACC)
TAIL_STOCK = _env("TAIL_STOCK", (True, True, True, True, True, False))
POOL_M12_FIRST = _env("POOL_M12_FIRST", 0)
TAIL_CHUNK = _env("TAIL_CHUNK", 1)  # last plane: per-mblock chain after its act pairs
LOAD_SPLIT0 = _env("LOAD_SPLIT0", 0)  # plane 0 loads: 2nd ring (1=Act, 2=Pool)
P0_HALF = _env("P0_HALF", 1)  # plane 0: half-plane loads/products/scans
ND_SPLIT = _env("ND_SPLIT", 1)  # split NUMDEN into num-op + den-op
SCAN_HALVES = _env("SCAN_HALVES", 0)  # split scans into per-half ops
SSM_ENG = _env("SSM_ENG", ("v",) * 6)  # ssm multiply engine per plane
DENRCP_ON = _env("DENRCP_ON", 0)  # fuse den + NR reciprocal in one op
SQ_EARLY = _env("SQ_EARLY", 0)  # emit Square pair right after PA/PB matmuls


# ---- custom DVE ops ------------------------------------------------------
def _register(name, spec, subdim=False):
    for op in dve_ops.OPS:
        if op.name == name:
            return op
    shas = {}
    for ver in ("v3", "v4"):
        s = DveOpSpec(name=name, opcode=0, uops=lower(spec, ver=ver),
                      rd1_en=True)
        shas[ver] = s.sha(ver)
    op = dve_ops.DveOp(name, spec, subdim=subdim, uops_sha=shas)
    dve_ops.OPS.append(op)
    dve_ops.CUSTOM_DVE_SPECS[name] = spec
    dve_ops._SUB_OPCODE_FOR_NAME[name] = (
        dve_ops._CUSTOM_DVE_ROW_BASE + len(dve_ops.OPS) - 1
    )
    return op


SQSCAN = _register(
    "SSIM_SQSCAN",
    Spec(
        body=scan(AluOp.ADD, sq(Src0) - sq(Src1)),
        reference=lambda in0, in1, c0, c1, c2: np.cumsum(
            in0.astype(np.float32) ** 2 - in1.astype(np.float32) ** 2,
            axis=-1,
        ),
    ),
)

NUMDEN = _register(
    "SSIM_NUMDEN",
    Spec(
        body=(Src0 + C0) * (Src1 - Src0),
        reference=lambda in0, in1, c0, c1, c2: (
            (in0.astype(np.float32) + c0) * (in1 - in0)
        ),
    ),
)

_not = Bin(AluOp.BITWISE_NOT, Src0, Src0)
_y0 = _not * C0
_y1 = _y0 * (C1 - Src0 * _y0)


def _ref_divacc(in0, in1, c0, c1, c2):
    nx = (~in0.astype(np.float32).view(np.int32)).view(np.float32)
    y0 = nx * c0
    y1 = y0 * (c1 - in0 * y0)
    o = (in1 * y1).astype(np.float32)
    return o, o.reshape(o.shape[0], -1).sum(-1, keepdims=True)


DIVACC = _register(
    "SSIM_DIVACC",
    Spec(body=Src1 * _y1, accum=AluOp.ADD, reference=_ref_divacc),
)
# den = (Src0 + c0) * (Src1 - Src0); out = NR1-approx(1/den)
_dn = (Src0 + C0) * (Src1 - Src0)
_dny0 = Bin(AluOp.BITWISE_NOT, _dn, _dn) * C1
_dny1 = _dny0 * (C2 - _dn * _dny0)


def _ref_denrcp(in0, in1, c0, c1, c2):
    dn = ((in0.astype(np.float32) + c0) * (in1 - in0)).astype(np.float32)
    nx = (~dn.view(np.int32)).view(np.float32)
    y0 = nx * c1
    return y0 * (c2 - dn * y0)


DENRCP = _register(
    "SSIM_DENRCP",
    Spec(body=_dny1, reference=_ref_denrcp),
)
_RC = dve_ops.RECIP_APPROX_FAST_CONSTS


def _build_weights() -> np.ndarray:
    w = np.zeros((NCLS, NP_, 128, 128), dtype=np.float32)
    vals = [W_A, W_B, -W_B]
    for idx, (m, k) in enumerate(_PAIRS):
        for i in range(128):
            for o in range(128):
                d = (128 * k + i) - (128 * m + o)
                if 0 <= d < WIN:
                    for c in range(NCLS):
                        w[c, idx, i, o] = vals[c]
    return np.ascontiguousarray(
        w.transpose(2, 0, 1, 3).reshape(128, NCLS * NP_, 128)
    ).astype(ml_dtypes.bfloat16)


def _act_recip(eng, out, in_):
    ins_l = [eng.lower_ap(in_)]
    for arg in (0.0, 1.0, 0.0):
        ins_l.append(mybir.ImmediateValue(dtype=mybir.dt.float32, value=arg))
    return eng.add_instruction(
        mybir.InstActivation(
            name=eng.bass.get_next_instruction_name(),
            func=ACTF.Reciprocal,
            ins=ins_l,
            outs=[eng.lower_ap(out)],
        )
    )


def _ktiles(m):
    return [m] if m == 3 else [m, m + 1]


def _kernel_body(ctx: ExitStack, tc: tile.TileContext, xy_d, wv_d, acc_d):
    nc = tc.nc

    singles = ctx.enter_context(tc.tile_pool(name="singles", bufs=1))
    xy_pool = ctx.enter_context(tc.tile_pool(name="xy", bufs=2))
    sd_pool = ctx.enter_context(tc.tile_pool(name="sd", bufs=SD_BUFS))
    h_pool = ctx.enter_context(tc.tile_pool(name="h", bufs=H_BUFS))
    ch_pool = ctx.enter_context(tc.tile_pool(name="ch", bufs=CH_BUFS))
    ch2_pool = ctx.enter_context(tc.tile_pool(name="ch2", bufs=CH2_BUFS))
    psum_pool = ctx.enter_context(tc.tile_pool(name="ps", bufs=2, space="PSUM"))

    wv_sb = singles.tile([128, NCLS * NP_, 128], BF16)
    acc_sb = singles.tile([128, 16], F32)
    nc.vector.memset(acc_sb[:], 0.0)

    def emit_load(p):
        xyb = xy_pool.tile([128, 2, NSEG, SEG], BF16, tag="xyb")
        nc.gpsimd.memset(xyb[:, :, :, 0:WIN], 0.0)
        e2 = nc.sync
        if LOAD_SPLIT or (LOAD_SPLIT0 and p == 0):
            e2 = nc.gpsimd if LOAD_SPLIT0 == 2 else nc.scalar
        if P0_HALF and p == 0:
            xr = xy_d[p].rearrange("j (s q) w -> q j s w", q=128)
            for h, eng in ((0, nc.sync), (1, e2)):
                for j in (0, 1):
                    eng.dma_start(
                        out=xyb[:, j, 2 * h: 2 * h + 2, WIN:SEG],
                        in_=xr[:, j, 2 * h: 2 * h + 2, :],
                    )
            return xyb
        for j, eng in ((0, nc.sync), (1, e2)):
            eng.dma_start(
                out=xyb[:, j, :, WIN:SEG],
                in_=xy_d[p, j].rearrange("(s q) w -> q s w", q=128),
            )
        return xyb

    def emit_products(p, xyb):
        # sm = x+y, dm = x-y over the full padded buffer (pads stay 0)
        sd = sd_pool.tile([128, 2, NSEG, SEG], BF16, tag="sd")
        xf = xyb[:].rearrange("q j s c -> q j (s c)")
        sf = sd[:].rearrange("q j s c -> q j (s c)")
        for j, op, cut in ((0, ALU.add, SM_DVE[p]), (1, ALU.subtract, DM_DVE[p])):
            if cut > 0:
                nc.vector.tensor_tensor(
                    out=sf[:, j, 0:cut], in0=xf[:, 0, 0:cut],
                    in1=xf[:, 1, 0:cut], op=op)
            if cut < BUF:
                nc.gpsimd.tensor_tensor(
                    out=sf[:, j, cut:BUF], in0=xf[:, 0, cut:BUF],
                    in1=xf[:, 1, cut:BUF], op=op)
        return sd

    def emit_prodscan_halves(p, xyb):
        # fill-path: per-half products + per-map-half scans so downstream
        # matmuls (which read segment ranges) start as early as possible
        sd = sd_pool.tile([128, 2, NSEG, SEG], BF16, tag="sd")
        hsd = h_pool.tile([128, 2, BUF], BF16, tag="hsd")
        hpr = h_pool.tile([128, 2, BUF], BF16, tag="hpr")
        HB = 2 * SEG  # 1046
        xf = xyb[:].rearrange("q j s c -> q j (s c)")
        sf = sd[:].rearrange("q j s c -> q j (s c)")
        for h in (0, 1):
            lo, hi = h * HB, (h + 1) * HB
            for j, op in ((0, ALU.add), (1, ALU.subtract)):
                nc.vector.tensor_tensor(
                    out=sf[:, j, lo:hi], in0=xf[:, 0, lo:hi],
                    in1=xf[:, 1, lo:hi], op=op)
            for j in (0, 1):
                nc.vector.tensor_tensor_scan(
                    out=hsd[:, j, lo + WIN:hi],
                    data0=sf[:, j, lo + WIN:hi],
                    data1=sf[:, j, lo:hi - WIN],
                    initial=0.0, op0=ALU.add, op1=ALU.subtract)
                nc.vector._custom_dve(
                    SQSCAN,
                    out=hpr[:, j, lo + WIN:hi],
                    in0=sf[:, j, lo + WIN:hi],
                    in1=sf[:, j, lo:hi - WIN])
        return sd, (hsd, hpr)

    def emit_scans(p, sd):
        hsd = h_pool.tile([128, 2, BUF], BF16, tag="hsd")
        hpr = h_pool.tile([128, 2, BUF], BF16, tag="hpr")
        if SCAN_HALVES:
            HB = 2 * SEG
            sf = sd[:].rearrange("q j s c -> q j (s c)")
            for h in (0, 1):
                lo, hi = h * HB, (h + 1) * HB
                for j in (0, 1):
                    nc.vector.tensor_tensor_scan(
                        out=hsd[:, j, lo + WIN:hi],
                        data0=sf[:, j, lo + WIN:hi],
                        data1=sf[:, j, lo:hi - WIN],
                        initial=0.0, op0=ALU.add, op1=ALU.subtract)
                    nc.vector._custom_dve(
                        SQSCAN,
                        out=hpr[:, j, lo + WIN:hi],
                        in0=sf[:, j, lo + WIN:hi],
                        in1=sf[:, j, lo:hi - WIN])
            return hsd, hpr
        sflat = sd[:].rearrange("q j s c -> q (j s c)")
        N2 = 2 * BUF
        nc.vector.tensor_tensor_scan(
            out=hsd[:].rearrange("q a b -> q (a b)")[:, WIN:N2],
            data0=sflat[:, WIN:N2],
            data1=sflat[:, 0:N2 - WIN],
            initial=0.0, op0=ALU.add, op1=ALU.subtract,
        )
        nc.vector._custom_dve(
            SQSCAN,
            out=hpr[:].rearrange("q a b -> q (a b)")[:, WIN:N2],
            in0=sflat[:, WIN:N2],
            in1=sflat[:, 0:N2 - WIN],
        )
        return hsd, hpr

    def emit_mm(p, hsd, hpr):
        TM = ch_pool.tile([128, 2, W4], BF16, tag="TM")
        CTV = ch_pool.tile([128, 2, W4], BF16, tag="CTV")
        co = 2 * (WIN - 1) + 1  # 21
        for m in range(4):
            sl = slice(OUT * m, OUT * (m + 1))
            pq = psum_pool.tile([128, 4, 512], F32, tag="pq")
            ks = _ktiles(m)
            # PA (bank0) from hs, PB (bank1) from hd: class 0
            for bi, j in ((0, 0), (1, 1)):
                for i, k in enumerate(ks):
                    nc.tensor.matmul(
                        pq[:, bi, 0:OUT],
                        wv_sb[:, 0 * NP_ + _WIDX[(m, k)], :],
                        hsd[:, j, SEG * k + co: SEG * k + SEG],
                        start=(i == 0), stop=(i == len(ks) - 1),
                    )
            if SQ_EARLY and (SQ_EARLY > 1 or p >= NPLANE - TAIL_CHUNK):
                nc.scalar.activation(
                    out=TM[:, :, sl], in_=pq[:, 0:2, 0:OUT],
                    func=ACTF.Square)
            # T (bank2) = wb*hss - wb*hdd ; V2 (bank3) = wb*hss + wb*hdd
            for bi, cjs in ((2, ((1, 0), (2, 1))), (3, ((1, 0), (1, 1)))):
                nmm = len(cjs) * len(ks)
                i = 0
                for cls, j in cjs:
                    for k in ks:
                        nc.tensor.matmul(
                            pq[:, bi, 0:OUT],
                            wv_sb[:, cls * NP_ + _WIDX[(m, k)], :],
                            hpr[:, j, SEG * k + co: SEG * k + SEG],
                            start=(i == 0), stop=(i == nmm - 1),
                        )
                        i += 1
            if not (SQ_EARLY and (SQ_EARLY > 1 or p >= NPLANE - TAIL_CHUNK)):
                nc.scalar.activation(
                    out=TM[:, :, sl], in_=pq[:, 0:2, 0:OUT], func=ACTF.Square)
            nc.scalar.activation(
                out=CTV[:, :, sl], in_=pq[:, 2:4, 0:OUT], func=ACTF.Copy,
                bias=float(2.0 * C2H))
            if p >= NPLANE - TAIL_CHUNK:
                emit_tail_chunk(p, m, sl, TM, CTV)
        return TM, CTV

    def emit_tail_chunk(p, m, sl, TM, CTV):
        # last plane: chain per mblock, overlapping later mblocks' matmuls
        mp = OUT - 3 * 128  # 118
        np_ = 128 if m < 3 else mp
        M12 = st[p]["M12c"]
        ND = st[p]["NDc"]
        scr = st[p]["SCRc"]
        nc.vector.tensor_tensor(out=M12[:, 0, sl], in0=TM[:, 0, sl],
                                in1=TM[:, 1, sl], op=ALU.subtract)
        nc.vector.tensor_tensor(out=M12[:, 1, sl], in0=TM[:, 0, sl],
                                in1=TM[:, 1, sl], op=ALU.add)
        nc.vector._custom_dve(
            NUMDEN, out=ND[:, :, sl], in0=M12[:, :, sl], in1=CTV[:, :, sl],
            s0=float(2.0 * C1H))
        col = 16 - 4 * (NPLANE - p) + m
        nc.vector._custom_dve(
            DIVACC, out=scr[:np_, sl], accum_out=acc_sb[:np_, col: col + 1],
            in0=ND[:np_, 1, sl], in1=ND[:np_, 0, sl],
            s0=_RC["s0"], s1=_RC["s1"])

    def emit_m12(p, TM, CTV):
        M12 = ch2_pool.tile([128, 2, W4], BF16, tag="M12")
        eng1 = nc.vector if M1_ENG[p] == "v" else nc.gpsimd
        eng2 = nc.vector if M2_ENG[p] == "v" else nc.gpsimd
        eng1.tensor_tensor(out=M12[:, 0, :], in0=TM[:, 0, :],
                           in1=TM[:, 1, :], op=ALU.subtract)
        eng2.tensor_tensor(out=M12[:, 1, :], in0=TM[:, 0, :],
                           in1=TM[:, 1, :], op=ALU.add)
        if DEN_STOCK[p]:
            q1 = ch2_pool.tile([128, W4], BF16, tag="q1")
            nc.vector.tensor_scalar(out=q1[:], in0=M12[:, 1, :],
                                    scalar1=float(2.0 * C1H), scalar2=None,
                                    op0=ALU.add)
            return M12, q1
        return M12, None

    def emit_chain(p, M12, q1, CTV):
        ND = ch2_pool.tile([128, 2, W4], BF16, tag="ND")
        if DEN_STOCK[p]:
            dn = ch2_pool.tile([128, W4], BF16, tag="dn")
            nc.gpsimd.tensor_tensor(out=dn[:], in0=CTV[:, 1, :],
                                    in1=M12[:, 1, :], op=ALU.subtract)
            nc.gpsimd.tensor_tensor(out=ND[:, 1, :], in0=q1[:], in1=dn[:],
                                    op=ALU.mult)
            nc.vector._custom_dve(
                NUMDEN, out=ND[:, 0, :], in0=M12[:, 0, :],
                in1=CTV[:, 0, :], s0=float(2.0 * C1H),
            )
        elif ND_SPLIT:
            nc.vector._custom_dve(
                NUMDEN, out=ND[:, 0, :], in0=M12[:, 0, :],
                in1=CTV[:, 0, :], s0=float(2.0 * C1H))
            if DENRCP_ON and TAIL_STOCK[p]:
                # ND[:,1,:] holds approx 1/den directly
                nc.vector._custom_dve(
                    DENRCP, out=ND[:, 1, :], in0=M12[:, 1, :],
                    in1=CTV[:, 1, :], s0=float(2.0 * C1H),
                    s1=_RC["s0"], imm2=float(_RC["s1"]))
            else:
                nc.vector._custom_dve(
                    NUMDEN, out=ND[:, 1, :], in0=M12[:, 1, :],
                    in1=CTV[:, 1, :], s0=float(2.0 * C1H))
        else:
            nc.vector._custom_dve(
                NUMDEN,
                out=ND[:].rearrange("q a b -> q (a b)"),
                in0=M12[:].rearrange("q a b -> q (a b)"),
                in1=CTV[:].rearrange("q a b -> q (a b)"),
                s0=float(2.0 * C1H),
            )
        return ND

    def emit_div(p, ND, TM):
        c0 = 3 * OUT  # 1506
        mp = OUT - 3 * 128  # 118
        scr_t = ch2_pool.tile([128, W4], BF16, tag="scr")
        scr = scr_t[:]
        if TAIL_STOCK[p]:
            if DENRCP_ON and ND_SPLIT:
                rcp = ND[:, 1, :]
            else:
                rcp_t = ch2_pool.tile([128, W4], BF16, tag="rcp")
                _act_recip(nc.scalar, rcp_t[:], ND[:, 1, :])
                rcp = rcp_t[:]
            ssm_eng = nc.vector if SSM_ENG[p] == "v" else nc.gpsimd
            ssm_eng.tensor_tensor(out=scr[:, 0:c0], in0=ND[:, 0, 0:c0],
                                  in1=rcp[:, 0:c0], op=ALU.mult)
            ssm_eng.tensor_tensor(out=scr[:mp, c0:W4],
                                  in0=ND[:mp, 0, c0:W4],
                                  in1=rcp[:mp, c0:W4], op=ALU.mult)
            nc.scalar.activation(
                out=ND[:, 1, 0:c0], in_=scr[:, 0:c0], func=ACTF.Copy,
                accum_out=acc_sb[:, 2 * p: 2 * p + 1])
            nc.scalar.activation(
                out=ND[:mp, 1, c0:W4], in_=scr[:mp, c0:W4], func=ACTF.Copy,
                accum_out=acc_sb[:mp, 2 * p + 1: 2 * p + 2])
            return
        nc.vector._custom_dve(
            DIVACC, out=scr[:, 0:c0], accum_out=acc_sb[:, 2 * p: 2 * p + 1],
            in0=ND[:, 1, 0:c0], in1=ND[:, 0, 0:c0],
            s0=_RC["s0"], s1=_RC["s1"],
        )
        nc.vector._custom_dve(
            DIVACC, out=scr[:mp, c0:W4],
            accum_out=acc_sb[:mp, 2 * p + 1: 2 * p + 2],
            in0=ND[:mp, 1, c0:W4], in1=ND[:mp, 0, c0:W4],
            s0=_RC["s0"], s1=_RC["s1"],
        )

    # ---- software pipeline ----
    s1_it = [p + 1 for p in range(NPLANE)]  # products + scans
    s2_it = [p + 2 for p in range(NPLANE)]  # mm + act pairs
    s3a_it = [p + 3 for p in range(NPLANE)]  # m1/m2
    s3b_it = [p + 4 for p in range(NPLANE)]  # numden + div
    s3b_it[NPLANE - 1] -= 1  # compress last plane's tail
    st = {p: {} for p in range(NPLANE)}
    for it in range(max(s3b_it) + 1):
        # ready work first: loads, then products+scans (deps one it old),
        # then mm, then the cross-engine chain stages.
        if it < NPLANE:
            st[it]["xyb"] = emit_load(it)
        if it == 0:
            nc.sync.dma_start(out=wv_sb[:], in_=wv_d)
        if POOL_M12_FIRST:
            for p in range(NPLANE):
                if s3a_it[p] == it and s3b_it[p] != it:
                    st[p]["M12"] = emit_m12(p, *st[p]["mm"])
        if P0_HALF and it == 1 and "h" not in st[0]:
            st[0]["sd"], st[0]["h"] = emit_prodscan_halves(0, st[0]["xyb"])
        if PROD_EARLY == 1 and it < NPLANE:
            st[it]["sd"] = emit_products(it, st[it]["xyb"])
        for p in range(NPLANE):
            if s1_it[p] == it:
                if P0_HALF and p == 0:
                    continue
                if not PROD_EARLY:
                    st[p]["sd"] = emit_products(p, st[p]["xyb"])
                st[p]["h"] = emit_scans(p, st[p]["sd"])
        if not POOL_M12_FIRST:
            for p in range(NPLANE):
                if s3a_it[p] == it and s3b_it[p] != it:
                    st[p]["M12"] = emit_m12(p, *st[p]["mm"])
        for p in range(NPLANE):
            if s3b_it[p] == it and s3a_it[p] != it:
                st[p]["ND"] = emit_chain(p, *st[p]["M12"], st[p]["mm"][1])
                emit_div(p, st[p]["ND"], st[p]["mm"][0])
        for p in range(NPLANE):
            if s2_it[p] == it:
                if p >= NPLANE - TAIL_CHUNK:
                    m12c = ch2_pool.tile([128, 2, W4], BF16, tag="M12")
                    ndc = ch2_pool.tile([128, 2, W4], BF16, tag="ND")
                    scrc = ch2_pool.tile([128, W4], BF16, tag="scr")
                    st[p]["M12c"] = m12c
                    st[p]["NDc"] = ndc
                    st[p]["SCRc"] = scrc
                st[p]["mm"] = emit_mm(p, *st[p]["h"])
        for p in range(NPLANE):
            if s3a_it[p] == it and s3b_it[p] == it:
                if p >= NPLANE - TAIL_CHUNK:
                    continue
                st[p]["M12"] = emit_m12(p, *st[p]["mm"])
                st[p]["ND"] = emit_chain(p, *st[p]["M12"], st[p]["mm"][1])
                emit_div(p, st[p]["ND"], st[p]["mm"][0])
        if PROD_EARLY == 2 and it < NPLANE and not (P0_HALF and it == 0):
            st[it]["sd"] = emit_products(it, st[it]["xyb"])

    nc.sync.dma_start(out=acc_d, in_=acc_sb[:])


_CACHE = {}


class _Bacc(bacc.Bacc):
    def insert_act_table_loads(self):
        import bass_rust as _br
        from concourse.hw_specs import get_activation_tables

        tables = [
            (n, (f if n == "reciprocal_and_small" else set()))
            for n, f in get_activation_tables(self.m.arch).items()
        ]
        _br.insert_act_table_loads(self, tables)


def _get_nc():
    if "nc" in _CACHE:
        return _CACHE["nc"]
    nc = _Bacc("TRN2", target_bir_lowering=False, debug=False)
    xy_d = nc.dram_tensor(
        "xy", [NPLANE, 2, IMG, IMG], BF16, kind="ExternalInput").ap()
    wv_d = nc.dram_tensor(
        "wv", [128, NCLS * NP_, 128], BF16, kind="ExternalInput").ap()
    acc_d = nc.dram_tensor("acc", [128, 16], F32, kind="ExternalOutput").ap()
    with tile.TileContext(nc) as tc, ExitStack() as ctx:
        _kernel_body(ctx, tc, xy_d, wv_d, acc_d)
    nc.compile()
    _CACHE["nc"] = nc
    return nc


def _run(x, y, trace=False, **kw):
    nc = _get_nc()
    wv = _build_weights()
    x = np.asarray(x, dtype=np.float32).astype(ml_dtypes.bfloat16)
    y = np.asarray(y, dtype=np.float32).astype(ml_dtypes.bfloat16)
    b_per = x.shape[0] // NCORES
    in_maps = []
    for c in range(NCORES):
        xs = x[c * b_per: (c + 1) * b_per].reshape(NPLANE, IMG, IMG)
        ys = y[c * b_per: (c + 1) * b_per].reshape(NPLANE, IMG, IMG)
        xy = np.ascontiguousarray(np.stack([xs, ys], axis=1))
        in_maps.append({"xy": xy, "wv": wv})
    res = bass_utils.run_bass_kernel_spmd(
        nc, in_maps, core_ids=list(range(NCORES)), trace=trace, **kw
    )
    total = 0.0
    for r in res.results:
        total += r["acc"].astype(np.float64).sum()
    mean = total / float(16 * 3 * OUT * OUT)
    out = np.float32(1.0 - mean)
    return out, res


def kernel(x, y):
    out, _ = _run(x, y, trace=False)
    return out


# revision 7
# speedup vs baseline: 1.0135x; 1.0055x over previous
"""SSIM loss kernel for Trainium2, v18: s/d basis + custom DVE ops.

Per core: 6 planes of 512x512. Host casts x,y to bf16 and stacks.
Per plane:
  sm=x+y, dm=x-y (TT, split DVE/Pool by column knob)
  scan1: stock merged scan over [sm,dm] -> hs,hd        (DVE)
  SQSCAN: custom scan h += Src0^2-Src1^2 over [sm,dm] -> hss,hdd (DVE)
  42 matmuls/plane: PA=w.hs, PB=w.hd, T=wb.hss-wb.hdd, V2=wb.(hss+hdd)
  Act: Square pair (PA,PB)->MS,MD ; Copy+2c2 pair (T,V2)->cT,cV
  m1=MS-MD, m2=MS+MD (TT, engine knobs)
  NUMDEN custom: [num,den] = (M12+2c1)*(CTV-M12) paired   (DVE)
  DIVACC custom: ssim=num*recip_nr1(den), accum -> acc    (DVE)
Host: 1 - sum(acc)/(48*502*502).
"""

import sys
from contextlib import ExitStack

import numpy as np

sys.path.insert(0, "/opt/trn_rl_repo")

import ml_dtypes  # noqa: E402

import concourse.bass as bass  # noqa: E402
import concourse.tile as tile  # noqa: E402
from concourse import bacc, bass_utils, mybir  # noqa: E402
from concourse import dve_ops  # noqa: E402
from concourse.dve_spec import (  # noqa: E402
    AluOp, Bin, C0, C1, C2, Spec, Src0, Src1, lower, scan, sq,
)
from concourse.dve_uop import DveOpSpec  # noqa: E402

F32 = mybir.dt.float32
BF16 = mybir.dt.bfloat16
ALU = mybir.AluOpType
ACTF = mybir.ActivationFunctionType

WIN = 11
IMG = 512
OUT = IMG - WIN + 1  # 502
SEG = WIN + IMG  # 523
NSEG = 4
BUF = NSEG * SEG  # 2092
NPLANE = 6
NCORES = 8
W4 = 4 * OUT  # 2008

C1C = (0.01 * 1.0) ** 2
C2C = (0.03 * 1.0) ** 2
G = 121.0 / 128.0
C1H = np.float32(C1C * G * G)
C2H = np.float32(C2C * G * G)
W_A = float(2.0 ** -7)
W_B = float(121.0 * 2.0 ** -14)
NCLS = 3

_PAIRS = [(0, 0), (0, 1), (1, 1), (1, 2), (2, 2), (2, 3), (3, 3)]
_WIDX = {mk: i for i, mk in enumerate(_PAIRS)}
NP_ = len(_PAIRS)

# ---- knobs ---------------------------------------------------------------
import os as _os


def _env(name, default):
    v = _os.environ.get("V18_" + name)
    if v is None:
        return default
    return eval(v)  # noqa: S307 - trusted local tuning knob


# per-plane: first *_DVE cols of sm/dm on DVE, rest on Pool
SM_DVE = _env("SM_DVE", (BUF, 1046, 1046, 1046, 1046, 1046))
DM_DVE = _env("DM_DVE", (BUF, 0, 0, 0, 0, 0))
# products emitted in the load iteration (Pool gets a stage of slack)
PROD_EARLY = _env("PROD_EARLY", 2)
# per-plane m1/m2 engine: v=DVE, p=Pool (last plane on DVE to cut drain)
M1_ENG = _env("M1_ENG", ("p", "p", "p", "p", "p", "v"))
M2_ENG = _env("M2_ENG", ("p", "p", "p", "p", "p", "v"))
# per-plane: den lane via stock ops on Pool/Act instead of fused NUMDEN
DEN_STOCK = _env("DEN_STOCK", (False,) * 6)
LOAD_SPLIT = _env("LOAD_SPLIT", 0)  # 1: y-load on Act DMA ring
SD_BUFS = _env("SD_BUFS", 3)
H_BUFS = _env("H_BUFS", 2)
CH_BUFS = _env("CH_BUFS", 3)
CH2_BUFS = _env("CH2_BUFS", 2)
# per-plane: tail via Act reciprocal + DVE 2x TT + Act accum (else DIVACC)
TAIL_STOCK = _env("TAIL_STOCK", (True, True, True, False, False, False))
POOL_M12_FIRST = _env("POOL_M12_FIRST", 0)
TAIL_CHUNK = _env("TAIL_CHUNK", 1)  # last plane: per-mblock chain after its act pairs
LOAD_SPLIT0 = _env("LOAD_SPLIT0", 0)  # plane 0 loads: 2nd ring (1=Act, 2=Pool)
P0_HALF = _env("P0_HALF", 1)  # plane 0: half-plane loads/products/scans
ND_SPLIT = _env("ND_SPLIT", 1)  # split NUMDEN into num-op + den-op
SCAN_HALVES = _env("SCAN_HALVES", 0)  # split scans into per-half ops
SSM_ENG = _env("SSM_ENG", ("v",) * 6)  # ssm multiply engine per plane
DENRCP_ON = _env("DENRCP_ON", 0)  # fuse den + NR reciprocal in one op
SQ_EARLY = _env("SQ_EARLY", 0)  # emit Square pair right after PA/PB matmuls


# ---- custom DVE ops ------------------------------------------------------
def _register(name, spec, subdim=False):
    for op in dve_ops.OPS:
        if op.name == name:
            return op
    shas = {}
    for ver in ("v3", "v4"):
        s = DveOpSpec(name=name, opcode=0, uops=lower(spec, ver=ver),
                      rd1_en=True)
        shas[ver] = s.sha(ver)
    op = dve_ops.DveOp(name, spec, subdim=subdim, uops_sha=shas)
    dve_ops.OPS.append(op)
    dve_ops.CUSTOM_DVE_SPECS[name] = spec
    dve_ops._SUB_OPCODE_FOR_NAME[name] = (
        dve_ops._CUSTOM_DVE_ROW_BASE + len(dve_ops.OPS) - 1
    )
    return op


SQSCAN = _register(
    "SSIM_SQSCAN",
    Spec(
        body=scan(AluOp.ADD, sq(Src0) - sq(Src1)),
        reference=lambda in0, in1, c0, c1, c2: np.cumsum(
            in0.astype(np.float32) ** 2 - in1.astype(np.float32) ** 2,
            axis=-1,
        ),
    ),
)

NUMDEN = _register(
    "SSIM_NUMDEN",
    Spec(
        body=(Src0 + C0) * (Src1 - Src0),
        reference=lambda in0, in1, c0, c1, c2: (
            (in0.astype(np.float32) + c0) * (in1 - in0)
        ),
    ),
)

_not = Bin(AluOp.BITWISE_NOT, Src0, Src0)
_y0 = _not * C0
_y1 = _y0 * (C1 - Src0 * _y0)


def _ref_divacc(in0, in1, c0, c1, c2):
    nx = (~in0.astype(np.float32).view(np.int32)).view(np.float32)
    y0 = nx * c0
    y1 = y0 * (c1 - in0 * y0)
    o = (in1 * y1).astype(np.float32)
    return o, o.reshape(o.shape[0], -1).sum(-1, keepdims=True)


DIVACC = _register(
    "SSIM_DIVACC",
    Spec(body=Src1 * _y1, accum=AluOp.ADD, reference=_ref_divacc),
)
# den = (Src0 + c0) * (Src1 - Src0); out = NR1-approx(1/den)
_dn = (Src0 + C0) * (Src1 - Src0)
_dny0 = Bin(AluOp.BITWISE_NOT, _dn, _dn) * C1
_dny1 = _dny0 * (C2 - _dn * _dny0)


def _ref_denrcp(in0, in1, c0, c1, c2):
    dn = ((in0.astype(np.float32) + c0) * (in1 - in0)).astype(np.float32)
    nx = (~dn.view(np.int32)).view(np.float32)
    y0 = nx * c1
    return y0 * (c2 - dn * y0)


DENRCP = _register(
    "SSIM_DENRCP",
    Spec(body=_dny1, reference=_ref_denrcp),
)
_RC = dve_ops.RECIP_APPROX_FAST_CONSTS


def _build_weights() -> np.ndarray:
    w = np.zeros((NCLS, NP_, 128, 128), dtype=np.float32)
    vals = [W_A, W_B, -W_B]
    for idx, (m, k) in enumerate(_PAIRS):
        for i in range(128):
            for o in range(128):
                d = (128 * k + i) - (128 * m + o)
                if 0 <= d < WIN:
                    for c in range(NCLS):
                        w[c, idx, i, o] = vals[c]
    return np.ascontiguousarray(
        w.transpose(2, 0, 1, 3).reshape(128, NCLS * NP_, 128)
    ).astype(ml_dtypes.bfloat16)


def _act_recip(eng, out, in_):
    ins_l = [eng.lower_ap(in_)]
    for arg in (0.0, 1.0, 0.0):
        ins_l.append(mybir.ImmediateValue(dtype=mybir.dt.float32, value=arg))
    return eng.add_instruction(
        mybir.InstActivation(
            name=eng.bass.get_next_instruction_name(),
            func=ACTF.Reciprocal,
            ins=ins_l,
            outs=[eng.lower_ap(out)],
        )
    )


def _ktiles(m):
    return [m] if m == 3 else [m, m + 1]


def _kernel_body(ctx: ExitStack, tc: tile.TileContext, xy_d, wv_d, acc_d):
    nc = tc.nc

    singles = ctx.enter_context(tc.tile_pool(name="singles", bufs=1))
    xy_pool = ctx.enter_context(tc.tile_pool(name="xy", bufs=2))
    sd_pool = ctx.enter_context(tc.tile_pool(name="sd", bufs=SD_BUFS))
    h_pool = ctx.enter_context(tc.tile_pool(name="h", bufs=H_BUFS))
    ch_pool = ctx.enter_context(tc.tile_pool(name="ch", bufs=CH_BUFS))
    ch2_pool = ctx.enter_context(tc.tile_pool(name="ch2", bufs=CH2_BUFS))
    psum_pool = ctx.enter_context(tc.tile_pool(name="ps", bufs=2, space="PSUM"))

    wv_sb = singles.tile([128, NCLS * NP_, 128], BF16)
    acc_sb = singles.tile([128, 16], F32)
    nc.vector.memset(acc_sb[:], 0.0)

    def emit_load(p):
        xyb = xy_pool.tile([128, 2, NSEG, SEG], BF16, tag="xyb")
        nc.gpsimd.memset(xyb[:, :, :, 0:WIN], 0.0)
        e2 = nc.sync
        if LOAD_SPLIT or (LOAD_SPLIT0 and p == 0):
            e2 = nc.gpsimd if LOAD_SPLIT0 == 2 else nc.scalar
        if P0_HALF and p == 0:
            xr = xy_d[p].rearrange("j (s q) w -> q j s w", q=128)
            for h, eng in ((0, nc.sync), (1, e2)):
                for j in (0, 1):
                    eng.dma_start(
                        out=xyb[:, j, 2 * h: 2 * h + 2, WIN:SEG],
                        in_=xr[:, j, 2 * h: 2 * h + 2, :],
                    )
            return xyb
        for j, eng in ((0, nc.sync), (1, e2)):
            eng.dma_start(
                out=xyb[:, j, :, WIN:SEG],
                in_=xy_d[p, j].rearrange("(s q) w -> q s w", q=128),
            )
        return xyb

    def emit_products(p, xyb):
        # sm = x+y, dm = x-y over the full padded buffer (pads stay 0)
        sd = sd_pool.tile([128, 2, NSEG, SEG], BF16, tag="sd")
        xf = xyb[:].rearrange("q j s c -> q j (s c)")
        sf = sd[:].rearrange("q j s c -> q j (s c)")
        for j, op, cut in ((0, ALU.add, SM_DVE[p]), (1, ALU.subtract, DM_DVE[p])):
            if cut > 0:
                nc.vector.tensor_tensor(
                    out=sf[:, j, 0:cut], in0=xf[:, 0, 0:cut],
                    in1=xf[:, 1, 0:cut], op=op)
            if cut < BUF:
                nc.gpsimd.tensor_tensor(
                    out=sf[:, j, cut:BUF], in0=xf[:, 0, cut:BUF],
                    in1=xf[:, 1, cut:BUF], op=op)
        return sd

    def emit_prodscan_halves(p, xyb):
        # fill-path: per-half products + per-map-half scans so downstream
        # matmuls (which read segment ranges) start as early as possible
        sd = sd_pool.tile([128, 2, NSEG, SEG], BF16, tag="sd")
        hsd = h_pool.tile([128, 2, BUF], BF16, tag="hsd")
        hpr = h_pool.tile([128, 2, BUF], BF16, tag="hpr")
        HB = 2 * SEG  # 1046
        xf = xyb[:].rearrange("q j s c -> q j (s c)")
        sf = sd[:].rearrange("q j s c -> q j (s c)")
        for h in (0, 1):
            lo, hi = h * HB, (h + 1) * HB
            for j, op in ((0, ALU.add), (1, ALU.subtract)):
                nc.vector.tensor_tensor(
                    out=sf[:, j, lo:hi], in0=xf[:, 0, lo:hi],
                    in1=xf[:, 1, lo:hi], op=op)
            for j in (0, 1):
                nc.vector.tensor_tensor_scan(
                    out=hsd[:, j, lo + WIN:hi],
                    data0=sf[:, j, lo + WIN:hi],
                    data1=sf[:, j, lo:hi - WIN],
                    initial=0.0, op0=ALU.add, op1=ALU.subtract)
                nc.vector._custom_dve(
                    SQSCAN,
                    out=hpr[:, j, lo + WIN:hi],
                    in0=sf[:, j, lo + WIN:hi],
                    in1=sf[:, j, lo:hi - WIN])
        return sd, (hsd, hpr)

    def emit_scans(p, sd):
        hsd = h_pool.tile([128, 2, BUF], BF16, tag="hsd")
        hpr = h_pool.tile([128, 2, BUF], BF16, tag="hpr")
        if SCAN_HALVES:
            HB = 2 * SEG
            sf = sd[:].rearrange("q j s c -> q j (s c)")
            for h in (0, 1):
                lo, hi = h * HB, (h + 1) * HB
                for j in (0, 1):
                    nc.vector.tensor_tensor_scan(
                        out=hsd[:, j, lo + WIN:hi],
                        data0=sf[:, j, lo + WIN:hi],
                        data1=sf[:, j, lo:hi - WIN],
                        initial=0.0, op0=ALU.add, op1=ALU.subtract)
                    nc.vector._custom_dve(
                        SQSCAN,
                        out=hpr[:, j, lo + WIN:hi],
                        in0=sf[:, j, lo + WIN:hi],
                        in1=sf[:, j, lo:hi - WIN])
            return hsd, hpr
        sflat = sd[:].rearrange("q j s c -> q (j s c)")
        N2 = 2 * BUF
        nc.vector.tensor_tensor_scan(
            out=hsd[:].rearrange("q a b -> q (a b)")[:, WIN:N2],
            data0=sflat[:, WIN:N2],
            data1=sflat[:, 0:N2 - WIN],
            initial=0.0, op0=ALU.add, op1=ALU.subtract,
        )
        nc.vector._custom_dve(
            SQSCAN,
            out=hpr[:].rearrange("q a b -> q (a b)")[:, WIN:N2],
            in0=sflat[:, WIN:N2],
            in1=sflat[:, 0:N2 - WIN],
        )
        return hsd, hpr

    def emit_mm(p, hsd, hpr):
        TM = ch_pool.tile([128, 2, W4], BF16, tag="TM")
        CTV = ch_pool.tile([128, 2, W4], BF16, tag="CTV")
        co = 2 * (WIN - 1) + 1  # 21
        for m in range(4):
            sl = slice(OUT * m, OUT * (m + 1))
            pq = psum_pool.tile([128, 4, 512], F32, tag="pq")
            ks = _ktiles(m)
            # PA (bank0) from hs, PB (bank1) from hd: class 0
            for bi, j in ((0, 0), (1, 1)):
                for i, k in enumerate(ks):
                    nc.tensor.matmul(
                        pq[:, bi, 0:OUT],
                        wv_sb[:, 0 * NP_ + _WIDX[(m, k)], :],
                        hsd[:, j, SEG * k + co: SEG * k + SEG],
                        start=(i == 0), stop=(i == len(ks) - 1),
                    )
            if SQ_EARLY and (SQ_EARLY > 1 or p >= NPLANE - TAIL_CHUNK):
                nc.scalar.activation(
                    out=TM[:, :, sl], in_=pq[:, 0:2, 0:OUT],
                    func=ACTF.Square)
            # T (bank2) = wb*hss - wb*hdd ; V2 (bank3) = wb*hss + wb*hdd
            for bi, cjs in ((2, ((1, 0), (2, 1))), (3, ((1, 0), (1, 1)))):
                nmm = len(cjs) * len(ks)
                i = 0
                for cls, j in cjs:
                    for k in ks:
                        nc.tensor.matmul(
                            pq[:, bi, 0:OUT],
                            wv_sb[:, cls * NP_ + _WIDX[(m, k)], :],
                            hpr[:, j, SEG * k + co: SEG * k + SEG],
                            start=(i == 0), stop=(i == nmm - 1),
                        )
                        i += 1
            if not (SQ_EARLY and (SQ_EARLY > 1 or p >= NPLANE - TAIL_CHUNK)):
                nc.scalar.activation(
                    out=TM[:, :, sl], in_=pq[:, 0:2, 0:OUT], func=ACTF.Square)
            nc.scalar.activation(
                out=CTV[:, :, sl], in_=pq[:, 2:4, 0:OUT], func=ACTF.Copy,
                bias=float(2.0 * C2H))
            if p >= NPLANE - TAIL_CHUNK:
                emit_tail_chunk(p, m, sl, TM, CTV)
        return TM, CTV

    def emit_tail_chunk(p, m, sl, TM, CTV):
        # last plane: chain per mblock, overlapping later mblocks' matmuls
        mp = OUT - 3 * 128  # 118
        np_ = 128 if m < 3 else mp
        M12 = st[p]["M12c"]
        ND = st[p]["NDc"]
        scr = st[p]["SCRc"]
        nc.vector.tensor_tensor(out=M12[:, 0, sl], in0=TM[:, 0, sl],
                                in1=TM[:, 1, sl], op=ALU.subtract)
        nc.vector.tensor_tensor(out=M12[:, 1, sl], in0=TM[:, 0, sl],
                                in1=TM[:, 1, sl], op=ALU.add)
        nc.vector._custom_dve(
            NUMDEN, out=ND[:, :, sl], in0=M12[:, :, sl], in1=CTV[:, :, sl],
            s0=float(2.0 * C1H))
        col = 16 - 4 * (NPLANE - p) + m
        nc.vector._custom_dve(
            DIVACC, out=scr[:np_, sl], accum_out=acc_sb[:np_, col: col + 1],
            in0=ND[:np_, 1, sl], in1=ND[:np_, 0, sl],
            s0=_RC["s0"], s1=_RC["s1"])

    def emit_m12(p, TM, CTV):
        M12 = ch2_pool.tile([128, 2, W4], BF16, tag="M12")
        eng1 = nc.vector if M1_ENG[p] == "v" else nc.gpsimd
        eng2 = nc.vector if M2_ENG[p] == "v" else nc.gpsimd
        eng1.tensor_tensor(out=M12[:, 0, :], in0=TM[:, 0, :],
                           in1=TM[:, 1, :], op=ALU.subtract)
        eng2.tensor_tensor(out=M12[:, 1, :], in0=TM[:, 0, :],
                           in1=TM[:, 1, :], op=ALU.add)
        if DEN_STOCK[p]:
            q1 = ch2_pool.tile([128, W4], BF16, tag="q1")
            nc.vector.tensor_scalar(out=q1[:], in0=M12[:, 1, :],
                                    scalar1=float(2.0 * C1H), scalar2=None,
                                    op0=ALU.add)
            return M12, q1
        return M12, None

    def emit_chain(p, M12, q1, CTV):
        ND = ch2_pool.tile([128, 2, W4], BF16, tag="ND")
        if DEN_STOCK[p]:
            dn = ch2_pool.tile([128, W4], BF16, tag="dn")
            nc.gpsimd.tensor_tensor(out=dn[:], in0=CTV[:, 1, :],
                                    in1=M12[:, 1, :], op=ALU.subtract)
            nc.gpsimd.tensor_tensor(out=ND[:, 1, :], in0=q1[:], in1=dn[:],
                                    op=ALU.mult)
            nc.vector._custom_dve(
                NUMDEN, out=ND[:, 0, :], in0=M12[:, 0, :],
                in1=CTV[:, 0, :], s0=float(2.0 * C1H),
            )
        elif ND_SPLIT:
            nc.vector._custom_dve(
                NUMDEN, out=ND[:, 0, :], in0=M12[:, 0, :],
                in1=CTV[:, 0, :], s0=float(2.0 * C1H))
            if DENRCP_ON and TAIL_STOCK[p]:
                # ND[:,1,:] holds approx 1/den directly
                nc.vector._custom_dve(
                    DENRCP, out=ND[:, 1, :], in0=M12[:, 1, :],
                    in1=CTV[:, 1, :], s0=float(2.0 * C1H),
                    s1=_RC["s0"], imm2=float(_RC["s1"]))
            else:
                nc.vector._custom_dve(
                    NUMDEN, out=ND[:, 1, :], in0=M12[:, 1, :],
                    in1=CTV[:, 1, :], s0=float(2.0 * C1H))
        else:
            nc.vector._custom_dve(
                NUMDEN,
                out=ND[:].rearrange("q a b -> q (a b)"),
                in0=M12[:].rearrange("q a b -> q (a b)"),
                in1=CTV[:].rearrange("q a b -> q (a b)"),
                s0=float(2.0 * C1H),
            )
        return ND

    def emit_div(p, ND, TM):
        c0 = 3 * OUT  # 1506
        mp = OUT - 3 * 128  # 118
        scr_t = ch2_pool.tile([128, W4], BF16, tag="scr")
        scr = scr_t[:]
        if TAIL_STOCK[p]:
            if DENRCP_ON and ND_SPLIT:
                rcp = ND[:, 1, :]
            else:
                rcp_t = ch2_pool.tile([128, W4], BF16, tag="rcp")
                _act_recip(nc.scalar, rcp_t[:], ND[:, 1, :])
                rcp = rcp_t[:]
            ssm_eng = nc.vector if SSM_ENG[p] == "v" else nc.gpsimd
            ssm_eng.tensor_tensor(out=scr[:, 0:c0], in0=ND[:, 0, 0:c0],
                                  in1=rcp[:, 0:c0], op=ALU.mult)
            ssm_eng.tensor_tensor(out=scr[:mp, c0:W4],
                                  in0=ND[:mp, 0, c0:W4],
                                  in1=rcp[:mp, c0:W4], op=ALU.mult)
            nc.scalar.activation(
                out=ND[:, 1, 0:c0], in_=scr[:, 0:c0], func=ACTF.Copy,
                accum_out=acc_sb[:, 2 * p: 2 * p + 1])
            nc.scalar.activation(
                out=ND[:mp, 1, c0:W4], in_=scr[:mp, c0:W4], func=ACTF.Copy,
                accum_out=acc_sb[:mp, 2 * p + 1: 2 * p + 2])
            return
        nc.vector._custom_dve(
            DIVACC, out=scr[:, 0:c0], accum_out=acc_sb[:, 2 * p: 2 * p + 1],
            in0=ND[:, 1, 0:c0], in1=ND[:, 0, 0:c0],
            s0=_RC["s0"], s1=_RC["s1"],
        )
        nc.vector._custom_dve(
            DIVACC, out=scr[:mp, c0:W4],
            accum_out=acc_sb[:mp, 2 * p + 1: 2 * p + 2],
            in0=ND[:mp, 1, c0:W4], in1=ND[:mp, 0, c0:W4],
            s0=_RC["s0"], s1=_RC["s1"],
        )

    # ---- software pipeline ----
    s1_it = [p + 1 for p in range(NPLANE)]  # products + scans
    s2_it = [p + 2 for p in range(NPLANE)]  # mm + act pairs
    s3a_it = [p + 3 for p in range(NPLANE)]  # m1/m2
    s3b_it = [p + 4 for p in range(NPLANE)]  # numden + div
    s3b_it[NPLANE - 1] -= 1  # compress last plane's tail
    st = {p: {} for p in range(NPLANE)}
    for it in range(max(s3b_it) + 1):
        # ready work first: loads, then products+scans (deps one it old),
        # then mm, then the cross-engine chain stages.
        if it < NPLANE:
            st[it]["xyb"] = emit_load(it)
        if it == 0:
            nc.sync.dma_start(out=wv_sb[:], in_=wv_d)
        if POOL_M12_FIRST:
            for p in range(NPLANE):
                if s3a_it[p] == it and s3b_it[p] != it:
                    st[p]["M12"] = emit_m12(p, *st[p]["mm"])
        if P0_HALF and it == 1 and "h" not in st[0]:
            st[0]["sd"], st[0]["h"] = emit_prodscan_halves(0, st[0]["xyb"])
        if PROD_EARLY == 1 and it < NPLANE:
            st[it]["sd"] = emit_products(it, st[it]["xyb"])
        for p in range(NPLANE):
            if s1_it[p] == it:
                if P0_HALF and p == 0:
                    continue
                if not PROD_EARLY:
                    st[p]["sd"] = emit_products(p, st[p]["xyb"])
                st[p]["h"] = emit_scans(p, st[p]["sd"])
        if not POOL_M12_FIRST:
            for p in range(NPLANE):
                if s3a_it[p] == it and s3b_it[p] != it:
                    st[p]["M12"] = emit_m12(p, *st[p]["mm"])
        for p in range(NPLANE):
            if s3b_it[p] == it and s3a_it[p] != it:
                st[p]["ND"] = emit_chain(p, *st[p]["M12"], st[p]["mm"][1])
                emit_div(p, st[p]["ND"], st[p]["mm"][0])
        for p in range(NPLANE):
            if s2_it[p] == it:
                if p >= NPLANE - TAIL_CHUNK:
                    m12c = ch2_pool.tile([128, 2, W4], BF16, tag="M12")
                    ndc = ch2_pool.tile([128, 2, W4], BF16, tag="ND")
                    scrc = ch2_pool.tile([128, W4], BF16, tag="scr")
                    st[p]["M12c"] = m12c
                    st[p]["NDc"] = ndc
                    st[p]["SCRc"] = scrc
                st[p]["mm"] = emit_mm(p, *st[p]["h"])
        for p in range(NPLANE):
            if s3a_it[p] == it and s3b_it[p] == it:
                if p >= NPLANE - TAIL_CHUNK:
                    continue
                st[p]["M12"] = emit_m12(p, *st[p]["mm"])
                st[p]["ND"] = emit_chain(p, *st[p]["M12"], st[p]["mm"][1])
                emit_div(p, st[p]["ND"], st[p]["mm"][0])
        if PROD_EARLY == 2 and it < NPLANE and not (P0_HALF and it == 0):
            st[it]["sd"] = emit_products(it, st[it]["xyb"])

    nc.sync.dma_start(out=acc_d, in_=acc_sb[:])


_CACHE = {}


class _Bacc(bacc.Bacc):
    def insert_act_table_loads(self):
        import bass_rust as _br
        from concourse.hw_specs import get_activation_tables

        tables = [
            (n, (f if n == "reciprocal_and_small" else set()))
            for n, f in get_activation_tables(self.m.arch).items()
        ]
        _br.insert_act_table_loads(self, tables)


def _get_nc():
    if "nc" in _CACHE:
        return _CACHE["nc"]
    nc = _Bacc("TRN2", target_bir_lowering=False, debug=False)
    xy_d = nc.dram_tensor(
        "xy", [NPLANE, 2, IMG, IMG], BF16, kind="ExternalInput").ap()
    wv_d = nc.dram_tensor(
        "wv", [128, NCLS * NP_, 128], BF16, kind="ExternalInput").ap()
    acc_d = nc.dram_tensor("acc", [128, 16], F32, kind="ExternalOutput").ap()
    with tile.TileContext(nc) as tc, ExitStack() as ctx:
        _kernel_body(ctx, tc, xy_d, wv_d, acc_d)
    nc.compile()
    _CACHE["nc"] = nc
    return nc


def _run(x, y, trace=False, **kw):
    nc = _get_nc()
    wv = _build_weights()
    x = np.asarray(x, dtype=np.float32).astype(ml_dtypes.bfloat16)
    y = np.asarray(y, dtype=np.float32).astype(ml_dtypes.bfloat16)
    b_per = x.shape[0] // NCORES
    in_maps = []
    for c in range(NCORES):
        xs = x[c * b_per: (c + 1) * b_per].reshape(NPLANE, IMG, IMG)
        ys = y[c * b_per: (c + 1) * b_per].reshape(NPLANE, IMG, IMG)
        xy = np.ascontiguousarray(np.stack([xs, ys], axis=1))
        in_maps.append({"xy": xy, "wv": wv})
    res = bass_utils.run_bass_kernel_spmd(
        nc, in_maps, core_ids=list(range(NCORES)), trace=trace, **kw
    )
    total = 0.0
    for r in res.results:
        total += r["acc"].astype(np.float64).sum()
    mean = total / float(16 * 3 * OUT * OUT)
    out = np.float32(1.0 - mean)
    return out, res


def kernel(x, y):
    out, _ = _run(x, y, trace=False)
    return out
